# revision 3
# baseline (speedup 1.0000x reference)
"""Causal self-attention (B=2, S=2048, D=768, H=12) on 8 trn2 NeuronCores.

Sharding: batch*heads = 24 head-instances -> 3 heads per core
(cores 0-3: batch 0, cores 4-7: batch 1; core c%4 owns heads 3*(c%4)..3*(c%4)+2).
Each core computes Q/K/V projections for its 192 output dims, causal
attention for its 3 heads, and a partial out-projection
ctx[2048,192] @ Wo[192,768].  Host sums the 4 partials per batch and adds
the bias terms (bo, and bv folded through Wo: softmax rows sum to 1, so
V+bv contributes exactly bv@Wo to every output row).

Device layouts (per core):
  xt   [768, 2048]  x[b]^T              bf16
  wq/wk/wv [768, 192] col slices        bf16
  wo   [192, 768]  row slice            bf16
  bq2/bk2 [192, 1] f32 (bias applied per-partition in the transposed
           Q^T/K^T layout during PSUM eviction)
  masks [4, 128, 512] multiplicative causal masks for diagonal-band tiles

Attention is computed in transposed score layout S^T[k,q] so that no PE
transposes are needed: S^T tile = K_tile @ Q_chunk^T (contraction over
head_dim on partitions), exp on ScalarE, then ctx^T accumulates via
V_aug^T @ expS^T where V_aug carries a ones column that yields the
softmax denominator for free.  Normalization happens per 64-row ctx^T
slab via a K=1 broadcast matmul of the reciprocal denominators.
"""

import sys

sys.path.insert(0, "/opt/trn_rl_repo")

import numpy as np
import ml_dtypes

import concourse.bass as bass
import concourse.bacc as bacc
import concourse.tile as tile
from concourse import mybir
from concourse import bass_utils

BF16 = ml_dtypes.bfloat16
F32 = np.float32

B, S, D, H, HD = 2, 2048, 768, 12, 64
N_CORES = 8
CPB = 4          # cores per batch element
HPC = 3          # heads per core
HS = HPC * HD    # 192 output dims per core
QCH = 512        # q chunk (columns per scores matmul)
KT = 128         # k tile
NCH = S // QCH   # 4
NKT = S // KT    # 16
NMT = S // 128   # 16 seq tiles
ND = D // 128    # 6 contraction tiles for projections

_prog_cache = {}


def _emit(nc, tc, t):
    f32 = mybir.dt.float32
    bf16 = mybir.dt.bfloat16
    Ident = mybir.ActivationFunctionType.Identity
    Exp = mybir.ActivationFunctionType.Exp
    mult = mybir.AluOpType.mult

    import contextlib

    with contextlib.ExitStack() as ctx:
        sb = ctx.enter_context(tc.tile_pool(name="sb", bufs=1))
        esp = ctx.enter_context(tc.tile_pool(name="esp", bufs=4))
        smp = ctx.enter_context(tc.tile_pool(name="smp", bufs=2))
        obp = ctx.enter_context(tc.tile_pool(name="obp", bufs=3))
        # PSUM budget (8 banks): proj 2 + st/outproj 3 + cx/bc 3
        pj = ctx.enter_context(tc.tile_pool(name="pj", bufs=2, space="PSUM"))
        ps = ctx.enter_context(tc.tile_pool(name="ps", bufs=3, space="PSUM"))
        pc = ctx.enter_context(tc.tile_pool(name="pc", bufs=3, space="PSUM"))

        # ---- load everything ----
        xt = sb.tile([128, ND, S], bf16)
        nc.sync.dma_start(xt[:], t["xt"].ap().rearrange("(n p) m -> p n m", p=128))
        wq = sb.tile([128, ND, HS], bf16)
        nc.sync.dma_start(wq[:], t["wq"].ap().rearrange("(n p) m -> p n m", p=128))
        wk = sb.tile([128, ND, HS], bf16)
        nc.sync.dma_start(wk[:], t["wk"].ap().rearrange("(n p) m -> p n m", p=128))
        wv = sb.tile([128, ND, HS], bf16)
        nc.sync.dma_start(wv[:], t["wv"].ap().rearrange("(n p) m -> p n m", p=128))
        wo_a = sb.tile([128, D], bf16)
        nc.sync.dma_start(wo_a[:], t["wo"].ap()[0:128, :])
        wo_b = sb.tile([64, D], bf16)
        nc.sync.dma_start(wo_b[:], t["wo"].ap()[128:HS, :])
        masks = sb.tile([128, 4, QCH], bf16)
        nc.sync.dma_start(masks[:], t["masks"].ap().rearrange("a p m -> p a m"))
        bqa = sb.tile([128, 1], mybir.dt.float32)
        nc.sync.dma_start(bqa[:], t["bq2"].ap()[0:128, :])
        bqb = sb.tile([64, 1], mybir.dt.float32)
        nc.sync.dma_start(bqb[:], t["bq2"].ap()[128:HS, :])
        bka = sb.tile([128, 1], mybir.dt.float32)
        nc.sync.dma_start(bka[:], t["bk2"].ap()[0:128, :])
        bkb = sb.tile([64, 1], mybir.dt.float32)
        nc.sync.dma_start(bkb[:], t["bk2"].ap()[128:HS, :])

        ones = sb.tile([65, 64], f32)
        nc.vector.memset(ones[64:65, :], 1.0)

        qt_a = sb.tile([128, S], bf16)   # heads 0,1 of Q^T
        qt_b = sb.tile([64, S], bf16)    # head 2 of Q^T
        kt_a = sb.tile([128, S], bf16)
        kt_b = sb.tile([64, S], bf16)
        vaug = sb.tile([128, NKT, HPC, 65], bf16)
        nc.vector.memset(vaug[:, :, :, 64:65], 1.0)
        ctxt_a = sb.tile([128, S], bf16)
        ctxt_b = sb.tile([64, S], bf16)

        # ---- K^T and Q^T projections:  dst = W^T @ x^T  [HS, S] ----
        for wt, ba, bb, da, db in (
            (wk, bka, bkb, kt_a, kt_b),
            (wq, bqa, bqb, qt_a, qt_b),
        ):
            for jc in range(NCH):
                for m in range(2):
                    P = 128 if m == 0 else 64
                    pt = pj.tile([128, QCH], f32, tag="proj", name="pt")
                    for kd in range(ND):
                        nc.tensor.matmul(
                            pt[0:P, :],
                            wt[:, kd, 128 * m : 128 * m + P],
                            xt[:, kd, QCH * jc : QCH * (jc + 1)],
                            start=(kd == 0),
                            stop=(kd == ND - 1),
                        )
                    dst = da if m == 0 else db
                    bias = ba if m == 0 else bb
                    nc.scalar.activation(
                        dst[0:P, QCH * jc : QCH * (jc + 1)],
                        pt[0:P, :],
                        Ident,
                        bias=bias[0:P, :],
                        scale=1.0,
                    )

        # ---- V projection in natural [seq, hd] layout (bv folded on host) ----
        for mt in range(NMT):
            pv = pj.tile([128, QCH], f32, tag="proj", name="pv")
            for kd in range(ND):
                nc.tensor.matmul(
                    pv[:, 0:HS],
                    xt[:, kd, 128 * mt : 128 * (mt + 1)],
                    wv[:, kd, :],
                    start=(kd == 0),
                    stop=(kd == ND - 1),
                )
            for h in range(HPC):
                nc.vector.tensor_copy(
                    vaug[:, mt, h, 0:64], pv[:, 64 * h : 64 * h + 64]
                )

        # ---- attention per q-chunk, per head ----
        for jc in range(NCH):
            nkt = (QCH // KT) * (jc + 1)
            for h in range(HPC):
                if h < 2:
                    row, qsrc, ksrc = 64 * h, qt_a, kt_a
                else:
                    row, qsrc, ksrc = 0, qt_b, kt_b
                q_ap = qsrc[row : row + 64, QCH * jc : QCH * (jc + 1)]
                cx = pc.tile([65, QCH], f32, tag="cx", name="cx")
                for kt in range(nkt):
                    st = ps.tile([128, QCH], f32, tag="st", name="st")
                    nc.tensor.matmul(
                        st[:],
                        ksrc[row : row + 64, KT * kt : KT * (kt + 1)],
                        q_ap,
                        start=True,
                        stop=True,
                    )
                    es = esp.tile([128, QCH], bf16, name="es")
                    nc.scalar.activation(es[:], st[:], Exp, scale=float(1.0 / np.sqrt(HD)))
                    di = kt - (QCH // KT) * jc
                    if di >= 0:
                        nc.vector.tensor_mul(es[:], es[:], masks[:, di, :])
                    nc.tensor.matmul(
                        cx[:],
                        vaug[:, kt, h, :],
                        es[:],
                        start=(kt == 0),
                        stop=(kt == nkt - 1),
                    )
                # normalize: ctx^T[0:64] * broadcast(1/den)
                rec = smp.tile([65, QCH], f32, tag="rec", name="rec")
                nc.vector.reciprocal(rec[64:65, :], cx[64:65, :])
                bc = pc.tile([64, QCH], f32, tag="cx", name="bc")
                nc.tensor.matmul(
                    bc[:], ones[64:65, :], rec[64:65, :], start=True, stop=True
                )
                bcs = smp.tile([64, QCH], f32, tag="bcs", name="bcs")
                nc.vector.tensor_copy(bcs[:], bc[:])
                dctx = ctxt_a if h < 2 else ctxt_b
                nc.vector.scalar_tensor_tensor(
                    out=dctx[row : row + 64, QCH * jc : QCH * (jc + 1)],
                    in0=cx[0:64, :],
                    scalar=1.0,
                    in1=bcs[:],
                    op0=mult,
                    op1=mult,
                )

            # ---- out-projection for the 4 seq tiles of this chunk ----
            for mt in range(4 * jc, 4 * (jc + 1)):
                ob = obp.tile([128, D], f32, name="ob")
                for nh in range(2):
                    op = ps.tile([128, QCH], f32, tag="st", name="op")
                    nc.tensor.matmul(
                        op[:, 0:384],
                        ctxt_a[:, 128 * mt : 128 * (mt + 1)],
                        wo_a[:, 384 * nh : 384 * (nh + 1)],
                        start=True,
                        stop=False,
                    )
                    nc.tensor.matmul(
                        op[:, 0:384],
                        ctxt_b[:, 128 * mt : 128 * (mt + 1)],
                        wo_b[:, 384 * nh : 384 * (nh + 1)],
                        start=False,
                        stop=True,
                    )
                    nc.vector.tensor_copy(
                        ob[:, 384 * nh : 384 * (nh + 1)], op[:, 0:384]
                    )
                nc.sync.dma_start(
                    t["outp"].ap()[128 * mt : 128 * (mt + 1), :], ob[:]
                )


def _build_program():
    nc = bacc.Bacc(
        "TRN2", target_bir_lowering=False, debug=False, num_devices=N_CORES
    )
    bf16 = mybir.dt.bfloat16
    f32 = mybir.dt.float32
    t = {
        "xt": nc.dram_tensor("xt", [D, S], bf16, kind="ExternalInput"),
        "wq": nc.dram_tensor("wq", [D, HS], bf16, kind="ExternalInput"),
        "wk": nc.dram_tensor("wk", [D, HS], bf16, kind="ExternalInput"),
        "wv": nc.dram_tensor("wv", [D, HS], bf16, kind="ExternalInput"),
        "wo": nc.dram_tensor("wo", [HS, D], bf16, kind="ExternalInput"),
        "bq2": nc.dram_tensor("bq2", [HS, 1], f32, kind="ExternalInput"),
        "bk2": nc.dram_tensor("bk2", [HS, 1], f32, kind="ExternalInput"),
        "masks": nc.dram_tensor("masks", [4, 128, QCH], bf16, kind="ExternalInput"),
        "outp": nc.dram_tensor("outp", [S, D], f32, kind="ExternalOutput"),
    }
    with tile.TileContext(nc) as tc:
        _emit(nc, tc, t)
    nc.compile()
    return nc


def _get_program():
    if "nc" not in _prog_cache:
        _prog_cache["nc"] = _build_program()
    return _prog_cache["nc"]


def _make_masks():
    kk = np.arange(KT)[:, None]
    qq = np.arange(QCH)[None, :]
    m = np.stack([(qq >= kk + KT * i) for i in range(4)]).astype(BF16)
    return m


def _in_maps(x, Wq, Wk, Wv, Wo, bq, bk):
    masks = _make_masks()
    xts = [np.ascontiguousarray(x[b].T).astype(BF16) for b in range(B)]
    maps = []
    for c in range(N_CORES):
        b, g = divmod(c, CPB)
        c0 = HS * g
        maps.append(
            {
                "xt": xts[b],
                "wq": np.ascontiguousarray(Wq[:, c0 : c0 + HS]).astype(BF16),
                "wk": np.ascontiguousarray(Wk[:, c0 : c0 + HS]).astype(BF16),
                "wv": np.ascontiguousarray(Wv[:, c0 : c0 + HS]).astype(BF16),
                "wo": np.ascontiguousarray(Wo[c0 : c0 + HS, :]).astype(BF16),
                "bq2": np.ascontiguousarray(bq[c0 : c0 + HS]).reshape(HS, 1).astype(F32),
                "bk2": np.ascontiguousarray(bk[c0 : c0 + HS]).reshape(HS, 1).astype(F32),
                "masks": masks,
            }
        )
    return maps


def kernel(x, Wq, bq, Wk, bk, Wv, bv, Wo, bo):
    x = np.asarray(x, F32)
    Wq = np.asarray(Wq, F32)
    Wk = np.asarray(Wk, F32)
    Wv = np.asarray(Wv, F32)
    Wo = np.asarray(Wo, F32)
    bq = np.asarray(bq, F32)
    bk = np.asarray(bk, F32)
    bv = np.asarray(bv, F32)
    bo = np.asarray(bo, F32)

    nc = _get_program()
    in_maps = _in_maps(x, Wq, Wk, Wv, Wo, bq, bk)

    res = bass_utils.run_bass_kernel_spmd(
        nc, in_maps, core_ids=list(range(N_CORES))
    )
    out = np.zeros((B, S, D), F32)
    for b in range(B):
        for g in range(CPB):
            out[b] += res.results[b * CPB + g]["outp"]
    out += (bv @ Wo + bo)[None, None, :]
    return out


# revision 5
# speedup vs baseline: 19.3631x; 19.3631x over previous
"""Causal self-attention (B=2, S=2048, D=768, H=12) on 8 trn2 NeuronCores.

Sharding: batch*heads = 24 head-instances -> 3 heads per core
(cores 0-3: batch 0, cores 4-7: batch 1; core c%4 owns heads 3*(c%4)..3*(c%4)+2).
Each core computes Q/K/V projections for its 192 output dims, causal
attention for its 3 heads, and a partial out-projection
ctx[2048,192] @ Wo[192,768].  Host sums the 4 partials per batch and adds
the bias terms (bo, and bv folded through Wo: softmax rows sum to 1, so
V+bv contributes exactly bv@Wo to every output row).

Device layouts (per core):
  xt   [768, 2048]  x[b]^T              bf16
  wq/wk/wv [768, 192] col slices        bf16
  wo   [192, 768]  row slice            bf16
  bq2/bk2 [192, 1] f32 (bias applied per-partition in the transposed
           Q^T/K^T layout during PSUM eviction)
  masks [4, 128, 512] multiplicative causal masks for diagonal-band tiles

Attention is computed in transposed score layout S^T[k,q] so that no PE
transposes are needed: S^T tile = K_tile @ Q_chunk^T (contraction over
head_dim on partitions), exp on ScalarE, then ctx^T accumulates via
V_aug^T @ expS^T where V_aug carries a ones column that yields the
softmax denominator for free.  Normalization happens per 64-row ctx^T
slab via a K=1 broadcast matmul of the reciprocal denominators.
"""

import sys

sys.path.insert(0, "/opt/trn_rl_repo")

import numpy as np
import ml_dtypes

import concourse.bass as bass
import concourse.bacc as bacc
import concourse.tile as tile
from concourse import mybir
from concourse import bass_utils

BF16 = ml_dtypes.bfloat16
F32 = np.float32

B, S, D, H, HD = 2, 2048, 768, 12, 64
N_CORES = 8
CPB = 4          # cores per batch element
HPC = 3          # heads per core
HS = HPC * HD    # 192 output dims per core
QCH = 512        # q chunk (columns per scores matmul)
KT = 128         # k tile
NCH = S // QCH   # 4
NKT = S // KT    # 16
NMT = S // 128   # 16 seq tiles
ND = D // 128    # 6 contraction tiles for projections

_prog_cache = {}


def _emit(nc, tc, t):
    f32 = mybir.dt.float32
    bf16 = mybir.dt.bfloat16
    Ident = mybir.ActivationFunctionType.Identity
    Exp = mybir.ActivationFunctionType.Exp
    mult = mybir.AluOpType.mult

    import contextlib

    with contextlib.ExitStack() as ctx:
        sb = ctx.enter_context(tc.tile_pool(name="sb", bufs=1))
        esp = ctx.enter_context(tc.tile_pool(name="esp", bufs=4))
        smp = ctx.enter_context(tc.tile_pool(name="smp", bufs=2))
        obp = ctx.enter_context(tc.tile_pool(name="obp", bufs=3))
        # PSUM budget (8 banks): proj 2 + st/outproj 3 + cx/bc 3
        pj = ctx.enter_context(tc.tile_pool(name="pj", bufs=2, space="PSUM"))
        ps = ctx.enter_context(tc.tile_pool(name="ps", bufs=3, space="PSUM"))
        pc = ctx.enter_context(tc.tile_pool(name="pc", bufs=3, space="PSUM"))

        # ---- load everything ----
        xt = sb.tile([128, ND, S], bf16)
        nc.sync.dma_start(xt[:], t["xt"].ap().rearrange("(n p) m -> p n m", p=128))
        wq = sb.tile([128, ND, HS], bf16)
        nc.sync.dma_start(wq[:], t["wq"].ap().rearrange("(n p) m -> p n m", p=128))
        wk = sb.tile([128, ND, HS], bf16)
        nc.sync.dma_start(wk[:], t["wk"].ap().rearrange("(n p) m -> p n m", p=128))
        wv = sb.tile([128, ND, HS], bf16)
        nc.sync.dma_start(wv[:], t["wv"].ap().rearrange("(n p) m -> p n m", p=128))
        wo_a = sb.tile([128, D], bf16)
        nc.sync.dma_start(wo_a[:], t["wo"].ap()[0:128, :])
        wo_b = sb.tile([64, D], bf16)
        nc.sync.dma_start(wo_b[:], t["wo"].ap()[128:HS, :])
        masks = sb.tile([128, 4, QCH], bf16)
        nc.sync.dma_start(masks[:], t["masks"].ap().rearrange("a p m -> p a m"))
        bqa = sb.tile([128, 1], mybir.dt.float32)
        nc.sync.dma_start(bqa[:], t["bq2"].ap()[0:128, :])
        bqb = sb.tile([64, 1], mybir.dt.float32)
        nc.sync.dma_start(bqb[:], t["bq2"].ap()[128:HS, :])
        bka = sb.tile([128, 1], mybir.dt.float32)
        nc.sync.dma_start(bka[:], t["bk2"].ap()[0:128, :])
        bkb = sb.tile([64, 1], mybir.dt.float32)
        nc.sync.dma_start(bkb[:], t["bk2"].ap()[128:HS, :])

        ones = sb.tile([65, 64], f32)
        nc.vector.memset(ones[64:65, :], 1.0)

        qt_a = sb.tile([128, S], bf16)   # heads 0,1 of Q^T
        qt_b = sb.tile([64, S], bf16)    # head 2 of Q^T
        kt_a = sb.tile([128, S], bf16)
        kt_b = sb.tile([64, S], bf16)
        vaug = sb.tile([128, NKT, HPC, 65], bf16)
        nc.vector.memset(vaug[:, :, :, 64:65], 1.0)
        ctxt_a = sb.tile([128, S], bf16)
        ctxt_b = sb.tile([64, S], bf16)

        # ---- K^T and Q^T projections:  dst = W^T @ x^T  [HS, S] ----
        for wt, ba, bb, da, db in (
            (wk, bka, bkb, kt_a, kt_b),
            (wq, bqa, bqb, qt_a, qt_b),
        ):
            for jc in range(NCH):
                for m in range(2):
                    P = 128 if m == 0 else 64
                    pt = pj.tile([128, QCH], f32, tag="proj", name="pt")
                    for kd in range(ND):
                        nc.tensor.matmul(
                            pt[0:P, :],
                            wt[:, kd, 128 * m : 128 * m + P],
                            xt[:, kd, QCH * jc : QCH * (jc + 1)],
                            start=(kd == 0),
                            stop=(kd == ND - 1),
                        )
                    dst = da if m == 0 else db
                    bias = ba if m == 0 else bb
                    nc.scalar.activation(
                        dst[0:P, QCH * jc : QCH * (jc + 1)],
                        pt[0:P, :],
                        Ident,
                        bias=bias[0:P, :],
                        scale=1.0,
                    )

        # ---- V projection in natural [seq, hd] layout (bv folded on host) ----
        for mt in range(NMT):
            pv = pj.tile([128, QCH], f32, tag="proj", name="pv")
            for kd in range(ND):
                nc.tensor.matmul(
                    pv[:, 0:HS],
                    xt[:, kd, 128 * mt : 128 * (mt + 1)],
                    wv[:, kd, :],
                    start=(kd == 0),
                    stop=(kd == ND - 1),
                )
            for h in range(HPC):
                nc.vector.tensor_copy(
                    vaug[:, mt, h, 0:64], pv[:, 64 * h : 64 * h + 64]
                )

        # ---- attention per q-chunk, per head ----
        for jc in range(NCH):
            nkt = (QCH // KT) * (jc + 1)
            for h in range(HPC):
                if h < 2:
                    row, qsrc, ksrc = 64 * h, qt_a, kt_a
                else:
                    row, qsrc, ksrc = 0, qt_b, kt_b
                q_ap = qsrc[row : row + 64, QCH * jc : QCH * (jc + 1)]
                cx = pc.tile([65, QCH], f32, tag="cx", name="cx")
                for kt in range(nkt):
                    st = ps.tile([128, QCH], f32, tag="st", name="st")
                    nc.tensor.matmul(
                        st[:],
                        ksrc[row : row + 64, KT * kt : KT * (kt + 1)],
                        q_ap,
                        start=True,
                        stop=True,
                    )
                    es = esp.tile([128, QCH], bf16, name="es")
                    nc.scalar.activation(es[:], st[:], Exp, scale=float(1.0 / np.sqrt(HD)))
                    di = kt - (QCH // KT) * jc
                    if di >= 0:
                        nc.vector.tensor_mul(es[:], es[:], masks[:, di, :])
                    nc.tensor.matmul(
                        cx[:],
                        vaug[:, kt, h, :],
                        es[:],
                        start=(kt == 0),
                        stop=(kt == nkt - 1),
                    )
                # normalize: ctx^T[0:64] * broadcast(1/den)
                rec = smp.tile([65, QCH], f32, tag="rec", name="rec")
                nc.vector.reciprocal(rec[64:65, :], cx[64:65, :])
                bc = pc.tile([64, QCH], f32, tag="cx", name="bc")
                nc.tensor.matmul(
                    bc[:], ones[64:65, :], rec[64:65, :], start=True, stop=True
                )
                bcs = smp.tile([64, QCH], f32, tag="bcs", name="bcs")
                nc.vector.tensor_copy(bcs[:], bc[:])
                dctx = ctxt_a if h < 2 else ctxt_b
                nc.vector.scalar_tensor_tensor(
                    out=dctx[row : row + 64, QCH * jc : QCH * (jc + 1)],
                    in0=cx[0:64, :],
                    scalar=1.0,
                    in1=bcs[:],
                    op0=mult,
                    op1=mult,
                )

            # ---- out-projection for the 4 seq tiles of this chunk ----
            for mt in range(4 * jc, 4 * (jc + 1)):
                ob = obp.tile([128, D], f32, name="ob")
                for nh in range(2):
                    op = ps.tile([128, QCH], f32, tag="st", name="op")
                    nc.tensor.matmul(
                        op[:, 0:384],
                        ctxt_a[:, 128 * mt : 128 * (mt + 1)],
                        wo_a[:, 384 * nh : 384 * (nh + 1)],
                        start=True,
                        stop=False,
                    )
                    nc.tensor.matmul(
                        op[:, 0:384],
                        ctxt_b[:, 128 * mt : 128 * (mt + 1)],
                        wo_b[:, 384 * nh : 384 * (nh + 1)],
                        start=False,
                        stop=True,
                    )
                    nc.vector.tensor_copy(
                        ob[:, 384 * nh : 384 * (nh + 1)], op[:, 0:384]
                    )
                nc.sync.dma_start(
                    t["outp"].ap()[128 * mt : 128 * (mt + 1), :], ob[:]
                )


def _build_program(repeat=1):
    nc = bacc.Bacc(
        "TRN2", target_bir_lowering=False, debug=False, num_devices=N_CORES
    )
    bf16 = mybir.dt.bfloat16
    f32 = mybir.dt.float32
    t = {
        "xt": nc.dram_tensor("xt", [D, S], bf16, kind="ExternalInput"),
        "wq": nc.dram_tensor("wq", [D, HS], bf16, kind="ExternalInput"),
        "wk": nc.dram_tensor("wk", [D, HS], bf16, kind="ExternalInput"),
        "wv": nc.dram_tensor("wv", [D, HS], bf16, kind="ExternalInput"),
        "wo": nc.dram_tensor("wo", [HS, D], bf16, kind="ExternalInput"),
        "bq2": nc.dram_tensor("bq2", [HS, 1], f32, kind="ExternalInput"),
        "bk2": nc.dram_tensor("bk2", [HS, 1], f32, kind="ExternalInput"),
        "masks": nc.dram_tensor("masks", [4, 128, QCH], bf16, kind="ExternalInput"),
        "outp": nc.dram_tensor("outp", [S, D], f32, kind="ExternalOutput"),
    }
    with tile.TileContext(nc) as tc:
        for _ in range(repeat):
            _emit(nc, tc, t)
    nc.compile()
    return nc


def _get_program(repeat=1):
    if repeat not in _prog_cache:
        _prog_cache[repeat] = _build_program(repeat)
    return _prog_cache[repeat]


def _make_masks():
    kk = np.arange(KT)[:, None]
    qq = np.arange(QCH)[None, :]
    m = np.stack([(qq >= kk + KT * i) for i in range(4)]).astype(BF16)
    return m


def _in_maps(x, Wq, Wk, Wv, Wo, bq, bk):
    masks = _make_masks()
    xts = [np.ascontiguousarray(x[b].T).astype(BF16) for b in range(B)]
    maps = []
    for c in range(N_CORES):
        b, g = divmod(c, CPB)
        c0 = HS * g
        maps.append(
            {
                "xt": xts[b],
                "wq": np.ascontiguousarray(Wq[:, c0 : c0 + HS]).astype(BF16),
                "wk": np.ascontiguousarray(Wk[:, c0 : c0 + HS]).astype(BF16),
                "wv": np.ascontiguousarray(Wv[:, c0 : c0 + HS]).astype(BF16),
                "wo": np.ascontiguousarray(Wo[c0 : c0 + HS, :]).astype(BF16),
                "bq2": np.ascontiguousarray(bq[c0 : c0 + HS]).reshape(HS, 1).astype(F32),
                "bk2": np.ascontiguousarray(bk[c0 : c0 + HS]).reshape(HS, 1).astype(F32),
                "masks": masks,
            }
        )
    return maps


def kernel(x, Wq, bq, Wk, bk, Wv, bv, Wo, bo):
    x = np.asarray(x, F32)
    Wq = np.asarray(Wq, F32)
    Wk = np.asarray(Wk, F32)
    Wv = np.asarray(Wv, F32)
    Wo = np.asarray(Wo, F32)
    bq = np.asarray(bq, F32)
    bk = np.asarray(bk, F32)
    bv = np.asarray(bv, F32)
    bo = np.asarray(bo, F32)

    nc = _get_program()
    in_maps = _in_maps(x, Wq, Wk, Wv, Wo, bq, bk)

    res = bass_utils.run_bass_kernel_spmd(
        nc, in_maps, core_ids=list(range(N_CORES))
    )
    out = np.zeros((B, S, D), F32)
    for b in range(B):
        for g in range(CPB):
            out[b] += res.results[b * CPB + g]["outp"]
    out += (bv @ Wo + bo)[None, None, :]
    return out


# revision 13
# speedup vs baseline: 21.2511x; 1.0975x over previous
"""Causal self-attention (B=2, S=2048, D=768, H=12) on 8 trn2 NeuronCores.

Sharding: batch*heads = 24 head-instances -> 3 heads per core
(cores 0-3: batch 0, cores 4-7: batch 1; core c%4 owns heads 3*(c%4)..3*(c%4)+2).
Each core computes Q/K/V projections for its 192 output dims, causal
attention for its 3 heads, and a partial out-projection
ctx[2048,192] @ Wo[192,768].  Host sums the 4 partials per batch and adds
the bias terms (bo, and bv folded through Wo: softmax rows sum to 1, so
V+bv contributes exactly bv@Wo to every output row).

Device layouts (per core):
  xt   [768, 2048]  x[b]^T              bf16
  wq/wk/wv [768, 192] col slices        bf16
  wo   [192, 768]  row slice            bf16
  bq2/bk2 [192, 1] f32 (bias applied per-partition in the transposed
           Q^T/K^T layout during PSUM eviction)
  masks [4, 128, 512] multiplicative causal masks for diagonal-band tiles

Attention is computed in transposed score layout S^T[k,q] so that no PE
transposes are needed: S^T tile = K_tile @ Q_chunk^T (contraction over
head_dim on partitions), exp on ScalarE, then ctx^T accumulates via
V_aug^T @ expS^T where V_aug carries a ones column that yields the
softmax denominator for free.  Normalization happens per 64-row ctx^T
slab via a K=1 broadcast matmul of the reciprocal denominators.
"""

import sys

sys.path.insert(0, "/opt/trn_rl_repo")

import numpy as np
import ml_dtypes

import concourse.bass as bass
import concourse.bacc as bacc
import concourse.tile as tile
from concourse import mybir
from concourse import bass_utils

BF16 = ml_dtypes.bfloat16
F32 = np.float32

B, S, D, H, HD = 2, 2048, 768, 12, 64
N_CORES = 8
CPB = 4          # cores per batch element
HPC = 3          # heads per core
HS = HPC * HD    # 192 output dims per core
QCH = 512        # q chunk (columns per scores matmul)
KT = 128         # k tile
NCH = S // QCH   # 4
NKT = S // KT    # 16
NMT = S // 128   # 16 seq tiles
ND = D // 128    # 6 contraction tiles for projections

_prog_cache = {}


def _emit(nc, tc, t):
    f32 = mybir.dt.float32
    bf16 = mybir.dt.bfloat16
    Ident = mybir.ActivationFunctionType.Identity
    Exp = mybir.ActivationFunctionType.Exp
    mult = mybir.AluOpType.mult

    import contextlib

    add = mybir.AluOpType.add

    with contextlib.ExitStack() as ctx:
        sb = ctx.enter_context(tc.tile_pool(name="sb", bufs=1))
        esp = ctx.enter_context(tc.tile_pool(name="esp", bufs=8))
        smp = ctx.enter_context(tc.tile_pool(name="smp", bufs=3))
        obp = ctx.enter_context(tc.tile_pool(name="obp", bufs=3))
        # PSUM budget (8 banks): "st" tag (proj/scores/outproj) 4 + "cx" 4
        ps = ctx.enter_context(tc.tile_pool(name="ps", bufs=4, space="PSUM"))
        pc = ctx.enter_context(tc.tile_pool(name="pc", bufs=4, space="PSUM"))

        # ---- load everything (xt split per contraction tile) ----
        xt = sb.tile([128, ND, S], bf16)
        xt_r = t["xt"].ap().rearrange("(n p) m -> p n m", p=128)
        for kd in range(ND):
            nc.sync.dma_start(xt[:, kd, :], xt_r[:, kd, :])
        wq = sb.tile([128, ND, HS], bf16)
        nc.sync.dma_start(wq[:], t["wq"].ap().rearrange("(n p) m -> p n m", p=128))
        wk = sb.tile([128, ND, HS], bf16)
        nc.sync.dma_start(wk[:], t["wk"].ap().rearrange("(n p) m -> p n m", p=128))
        wv = sb.tile([128, ND, HS], bf16)
        nc.sync.dma_start(wv[:], t["wv"].ap().rearrange("(n p) m -> p n m", p=128))
        wo_a = sb.tile([128, D], bf16)
        nc.sync.dma_start(wo_a[:], t["wo"].ap()[0:128, :])
        wo_b = sb.tile([64, D], bf16)
        nc.sync.dma_start(wo_b[:], t["wo"].ap()[128:HS, :])
        bqa = sb.tile([128, 1], f32)
        nc.sync.dma_start(bqa[:], t["bq2"].ap()[0:128, :])
        bqb = sb.tile([64, 1], f32)
        nc.sync.dma_start(bqb[:], t["bq2"].ap()[128:HS, :])
        bka = sb.tile([128, 1], f32)
        nc.sync.dma_start(bka[:], t["bk2"].ap()[0:128, :])
        bkb = sb.tile([64, 1], f32)
        nc.sync.dma_start(bkb[:], t["bk2"].ap()[128:HS, :])
        tri = sb.tile([128, KT], bf16)  # tri[k, q] = 1.0 if q >= k
        nc.sync.dma_start(tri[:], t["tri"].ap())

        ones = sb.tile([65, 64], f32)
        nc.vector.memset(ones[64:65, :], 1.0)

        qt_a = sb.tile([128, S], bf16)   # heads 0,1 of Q^T
        qt_b = sb.tile([64, S], bf16)    # head 2 of Q^T
        kt_a = sb.tile([128, S], bf16)
        kt_b = sb.tile([64, S], bf16)
        vaug = sb.tile([128, NKT, HPC, 65], bf16)
        nc.vector.memset(vaug[:, :, :, 64:65], 1.0)
        ctxt_a = sb.tile([128, S], bf16)
        ctxt_b = sb.tile([64, S], bf16)

        # ---- K^T / Q^T projections (evict on DVE with per-partition bias);
        #      V between them so early attention chunks unblock sooner ----
        def qk_proj(wt, ba, bb, da, db):
            for jc in range(NCH):
                for m in range(2):
                    P = 128 if m == 0 else 64
                    pt = ps.tile([128, QCH], f32, tag="st", name="pt")
                    for kd in range(ND):
                        nc.tensor.matmul(
                            pt[0:P, :],
                            wt[:, kd, 128 * m : 128 * m + P],
                            xt[:, kd, QCH * jc : QCH * (jc + 1)],
                            start=(kd == 0),
                            stop=(kd == ND - 1),
                        )
                    dst = da if m == 0 else db
                    bias = ba if m == 0 else bb
                    nc.vector.tensor_scalar(
                        out=dst[0:P, QCH * jc : QCH * (jc + 1)],
                        in0=pt[0:P, :],
                        scalar1=bias[0:P, :],
                        scalar2=None,
                        op0=add,
                    )

        qk_proj(wk, bka, bkb, kt_a, kt_b)

        # V in natural [seq, hd] layout (bv folded on host)
        for mt in range(NMT):
            pv = ps.tile([128, QCH], f32, tag="st", name="pv")
            for kd in range(ND):
                nc.tensor.matmul(
                    pv[:, 0:HS],
                    xt[:, kd, 128 * mt : 128 * (mt + 1)],
                    wv[:, kd, :],
                    start=(kd == 0),
                    stop=(kd == ND - 1),
                )
            for h in range(HPC):
                nc.vector.tensor_copy(
                    vaug[:, mt, h, 0:64], pv[:, 64 * h : 64 * h + 64]
                )

        qk_proj(wq, bqa, bqb, qt_a, qt_b)

        # ---- attention: 3 heads as parallel pipelines, ctx one k-step
        #      behind scores so PE never head-of-line blocks on ACT ----
        def head_aps(h):
            if h < 2:
                return 64 * h, qt_a, kt_a, ctxt_a
            return 0, qt_b, kt_b, ctxt_b

        scale = float(1.0 / np.sqrt(HD))
        for jc in range(NCH):
            nkt = (QCH // KT) * (jc + 1)
            cxs, ess = {}, {}

            def scores(kt, h):
                row, qsrc, ksrc, _ = head_aps(h)
                di = kt - (QCH // KT) * jc  # diagonal-band index
                lo = 128 * di if di > 0 else 0  # valid q cols start
                st = ps.tile([128, QCH], f32, tag="st", name="st")
                nc.tensor.matmul(
                    st[:, lo:QCH],
                    ksrc[row : row + 64, KT * kt : KT * (kt + 1)],
                    qsrc[row : row + 64, QCH * jc + lo : QCH * (jc + 1)],
                    start=True,
                    stop=True,
                )
                es = esp.tile([128, QCH], bf16, name="es")
                if lo > 0:
                    nc.gpsimd.memset(es[:, 0:lo], 0)
                nc.scalar.activation(es[:, lo:QCH], st[:, lo:QCH], Exp, scale=scale)
                if di >= 0:  # mask the triangular diagonal 128x128 block
                    nc.vector.tensor_mul(
                        es[:, lo : lo + KT], es[:, lo : lo + KT], tri[:]
                    )
                ess[(kt, h)] = es

            def ctxmm(kt, h):
                nc.tensor.matmul(
                    cxs[h][:],
                    vaug[:, kt, h, :],
                    ess.pop((kt, h))[:],
                    start=(kt == 0),
                    stop=(kt == nkt - 1),
                )

            for h in range(HPC):
                cxs[h] = pc.tile([65, QCH], f32, tag="cx", name="cx")
            for kt in range(nkt):
                for h in range(HPC):
                    scores(kt, h)
                if kt > 0:
                    for h in range(HPC):
                        ctxmm(kt - 1, h)
            for h in range(HPC):
                ctxmm(nkt - 1, h)

            # normalize: ctx^T[0:64] * broadcast(1/den)
            for h in range(HPC):
                row, _, _, dctx = head_aps(h)
                cx = cxs[h]
                rec = smp.tile([65, QCH], f32, tag="rec", name="rec")
                nc.vector.reciprocal(rec[64:65, :], cx[64:65, :])
                bc = pc.tile([64, QCH], f32, tag="cx", name="bc")
                nc.tensor.matmul(
                    bc[:], ones[64:65, :], rec[64:65, :], start=True, stop=True
                )
                bcs = smp.tile([64, QCH], f32, tag="bcs", name="bcs")
                nc.vector.tensor_copy(bcs[:], bc[:])
                nc.vector.scalar_tensor_tensor(
                    out=dctx[row : row + 64, QCH * jc : QCH * (jc + 1)],
                    in0=cx[0:64, :],
                    scalar=1.0,
                    in1=bcs[:],
                    op0=mult,
                    op1=mult,
                )

            # ---- out-projection for the 4 seq tiles of this chunk ----
            for mt in range(4 * jc, 4 * (jc + 1)):
                ob = obp.tile([128, D], f32, name="ob")
                for nh in range(2):
                    op = ps.tile([128, QCH], f32, tag="st", name="op")
                    nc.tensor.matmul(
                        op[:, 0:384],
                        ctxt_a[:, 128 * mt : 128 * (mt + 1)],
                        wo_a[:, 384 * nh : 384 * (nh + 1)],
                        start=True,
                        stop=False,
                    )
                    nc.tensor.matmul(
                        op[:, 0:384],
                        ctxt_b[:, 128 * mt : 128 * (mt + 1)],
                        wo_b[:, 384 * nh : 384 * (nh + 1)],
                        start=False,
                        stop=True,
                    )
                    nc.vector.tensor_copy(
                        ob[:, 384 * nh : 384 * (nh + 1)], op[:, 0:384]
                    )
                nc.sync.dma_start(
                    t["outp"].ap()[128 * mt : 128 * (mt + 1), :], ob[:]
                )


def _build_program(repeat=1):
    nc = bacc.Bacc(
        "TRN2", target_bir_lowering=False, debug=False, num_devices=N_CORES
    )
    bf16 = mybir.dt.bfloat16
    f32 = mybir.dt.float32
    t = {
        "xt": nc.dram_tensor("xt", [D, S], bf16, kind="ExternalInput"),
        "wq": nc.dram_tensor("wq", [D, HS], bf16, kind="ExternalInput"),
        "wk": nc.dram_tensor("wk", [D, HS], bf16, kind="ExternalInput"),
        "wv": nc.dram_tensor("wv", [D, HS], bf16, kind="ExternalInput"),
        "wo": nc.dram_tensor("wo", [HS, D], bf16, kind="ExternalInput"),
        "bq2": nc.dram_tensor("bq2", [HS, 1], f32, kind="ExternalInput"),
        "bk2": nc.dram_tensor("bk2", [HS, 1], f32, kind="ExternalInput"),
        "tri": nc.dram_tensor("tri", [128, KT], bf16, kind="ExternalInput"),
        "outp": nc.dram_tensor("outp", [S, D], f32, kind="ExternalOutput"),
    }
    with tile.TileContext(nc) as tc:
        for _ in range(repeat):
            _emit(nc, tc, t)
    nc.compile()
    return nc


def _get_program(repeat=1):
    if repeat not in _prog_cache:
        _prog_cache[repeat] = _build_program(repeat)
    return _prog_cache[repeat]


def _in_maps(x, Wq, Wk, Wv, Wo, bq, bk):
    xts = [np.ascontiguousarray(x[b].T).astype(BF16) for b in range(B)]
    tri = (np.arange(KT)[None, :] >= np.arange(128)[:, None]).astype(BF16)
    maps = []
    for c in range(N_CORES):
        b, g = divmod(c, CPB)
        c0 = HS * g
        maps.append(
            {
                "xt": xts[b],
                "wq": np.ascontiguousarray(Wq[:, c0 : c0 + HS]).astype(BF16),
                "wk": np.ascontiguousarray(Wk[:, c0 : c0 + HS]).astype(BF16),
                "wv": np.ascontiguousarray(Wv[:, c0 : c0 + HS]).astype(BF16),
                "wo": np.ascontiguousarray(Wo[c0 : c0 + HS, :]).astype(BF16),
                "bq2": np.ascontiguousarray(bq[c0 : c0 + HS]).reshape(HS, 1).astype(F32),
                "bk2": np.ascontiguousarray(bk[c0 : c0 + HS]).reshape(HS, 1).astype(F32),
                "tri": tri,
            }
        )
    return maps


def kernel(x, Wq, bq, Wk, bk, Wv, bv, Wo, bo):
    x = np.asarray(x, F32)
    Wq = np.asarray(Wq, F32)
    Wk = np.asarray(Wk, F32)
    Wv = np.asarray(Wv, F32)
    Wo = np.asarray(Wo, F32)
    bq = np.asarray(bq, F32)
    bk = np.asarray(bk, F32)
    bv = np.asarray(bv, F32)
    bo = np.asarray(bo, F32)

    nc = _get_program()
    in_maps = _in_maps(x, Wq, Wk, Wv, Wo, bq, bk)

    res = bass_utils.run_bass_kernel_spmd(
        nc, in_maps, core_ids=list(range(N_CORES))
    )
    out = np.zeros((B, S, D), F32)
    for b in range(B):
        for g in range(CPB):
            out[b] += res.results[b * CPB + g]["outp"]
    out += (bv @ Wo + bo)[None, None, :]
    return out


# revision 23
# speedup vs baseline: 28.5577x; 1.3438x over previous
"""Causal self-attention (B=2, S=2048, D=768, H=12) on 8 trn2 NeuronCores.

Sharding: batch*heads = 24 head-instances -> 3 heads per core
(cores 0-3: batch 0, cores 4-7: batch 1; core c%4 owns heads 3*(c%4)..3*(c%4)+2).
Each core computes Q/K/V projections for its 192 output dims, causal
attention for its 3 heads, and a partial out-projection
ctx[2048,192] @ Wo[192,768].  Host sums the 4 partials per batch and adds
the bias terms (bo, and bv folded through Wo: softmax rows sum to 1, so
V+bv contributes exactly bv@Wo to every output row).

Device layouts (per core):
  xt   [768, 2048]  x[b]^T              bf16
  wq/wk/wv [768, 192] col slices        bf16
  wo   [192, 768]  row slice            bf16
  bq2/bk2 [192, 1] f32 (bias applied per-partition in the transposed
           Q^T/K^T layout during PSUM eviction)
  masks [4, 128, 512] multiplicative causal masks for diagonal-band tiles

Attention is computed in transposed score layout S^T[k,q] so that no PE
transposes are needed: S^T tile = K_tile @ Q_chunk^T (contraction over
head_dim on partitions), exp on ScalarE, then ctx^T accumulates via
V_aug^T @ expS^T where V_aug carries a ones column that yields the
softmax denominator for free.  Normalization happens per 64-row ctx^T
slab via a K=1 broadcast matmul of the reciprocal denominators.
"""

import sys

sys.path.insert(0, "/opt/trn_rl_repo")

import numpy as np
import ml_dtypes

import concourse.bass as bass
import concourse.bacc as bacc
import concourse.tile as tile
from concourse import mybir
from concourse import bass_utils

BF16 = ml_dtypes.bfloat16
F32 = np.float32

B, S, D, H, HD = 2, 2048, 768, 12, 64
N_CORES = 8
CPB = 4          # cores per batch element
HPC = 3          # heads per core
HS = HPC * HD    # 192 output dims per core
QCH = 512        # q chunk (columns per scores matmul)
KT = 128         # k tile
NCH = S // QCH   # 4
NKT = S // KT    # 16
NMT = S // 128   # 16 seq tiles
ND = D // 128    # 6 contraction tiles for projections

_prog_cache = {}


def _emit(nc, tc, t):
    f32 = mybir.dt.float32
    bf16 = mybir.dt.bfloat16
    Ident = mybir.ActivationFunctionType.Identity
    Exp = mybir.ActivationFunctionType.Exp
    mult = mybir.AluOpType.mult

    import contextlib

    add = mybir.AluOpType.add

    with contextlib.ExitStack() as ctx:
        sb = ctx.enter_context(tc.tile_pool(name="sb", bufs=1))
        esp = ctx.enter_context(tc.tile_pool(name="esp", bufs=10))
        smp = ctx.enter_context(tc.tile_pool(name="smp", bufs=3))
        obp = ctx.enter_context(tc.tile_pool(name="obp", bufs=2))
        # PSUM budget (8 banks): "st" (proj+scores) 4 + "cx" 3 + "misc" (bc/op) 1
        ps = ctx.enter_context(tc.tile_pool(name="ps", bufs=4, space="PSUM"))
        pc = ctx.enter_context(tc.tile_pool(name="pc", bufs=3, space="PSUM"))
        pm = ctx.enter_context(tc.tile_pool(name="pm", bufs=1, space="PSUM"))

        # ---- loads, ordered by first use; wk/xt interleaved per kd so the
        #      K projection can start accumulating as tiles arrive ----
        bka = sb.tile([128, 1], f32)
        nc.sync.dma_start(bka[:], t["bk2"].ap()[0:128, :])
        bkb = sb.tile([64, 1], f32)
        nc.sync.dma_start(bkb[:], t["bk2"].ap()[128:HS, :])
        bqa = sb.tile([128, 1], f32)
        nc.sync.dma_start(bqa[:], t["bq2"].ap()[0:128, :])
        bqb = sb.tile([64, 1], f32)
        nc.sync.dma_start(bqb[:], t["bq2"].ap()[128:HS, :])
        tri = sb.tile([128, KT], bf16)  # tri[k, q] = 1.0 if q >= k
        nc.sync.dma_start(tri[:], t["tri"].ap())

        xt = sb.tile([128, ND, S], bf16)
        wk = sb.tile([128, ND, HS], bf16)
        xt_r = t["xt"].ap().rearrange("(n p) m -> p n m", p=128)
        wk_r = t["wk"].ap().rearrange("(n p) m -> p n m", p=128)
        for kd in range(ND):
            nc.sync.dma_start(wk[:, kd, :], wk_r[:, kd, :])
            nc.sync.dma_start(xt[:, kd, :], xt_r[:, kd, :])
        wq = sb.tile([128, ND, HS], bf16)
        nc.sync.dma_start(wq[:], t["wq"].ap().rearrange("(n p) m -> p n m", p=128))
        wv = sb.tile([128, ND, HS], bf16)
        nc.sync.dma_start(wv[:], t["wv"].ap().rearrange("(n p) m -> p n m", p=128))
        wo_a = sb.tile([128, D], bf16)
        nc.sync.dma_start(wo_a[:], t["wo"].ap()[0:128, :])
        wo_b = sb.tile([64, D], bf16)
        nc.sync.dma_start(wo_b[:], t["wo"].ap()[128:HS, :])

        ones = sb.tile([65, 64], f32)
        nc.vector.memset(ones[64:65, :], 1.0)

        qt_a = sb.tile([128, S], bf16)   # heads 0,1 of Q^T
        qt_b = sb.tile([64, S], bf16)    # head 2 of Q^T
        kt_a = sb.tile([128, S], bf16)
        kt_b = sb.tile([64, S], bf16)
        vaug = sb.tile([128, NKT, HPC, 65], bf16)
        nc.vector.memset(vaug[:, :, :, 64:65], 1.0)
        ctxt_a = sb.tile([128, S], bf16)
        ctxt_b = sb.tile([64, S], bf16)

        # ---- K^T projection helper: weights stationary per (m, kd), the
        #      given chunks' psums accumulate together ----
        def kt_proj(jcs):
            def run():
                for m in range(2):
                    P = 128 if m == 0 else 64
                    pts = {
                        jc: ps.tile([128, QCH], f32, tag="st", name="pt")
                        for jc in jcs
                    }
                    for kd in range(ND):
                        for jc in jcs:
                            nc.tensor.matmul(
                                pts[jc][0:P, :],
                                wk[:, kd, 128 * m : 128 * m + P],
                                xt[:, kd, QCH * jc : QCH * (jc + 1)],
                                start=(kd == 0),
                                stop=(kd == ND - 1),
                            )
                    dst = kt_a if m == 0 else kt_b
                    bias = bka if m == 0 else bkb
                    for jc in jcs:
                        nc.vector.tensor_scalar(
                            out=dst[0:P, QCH * jc : QCH * (jc + 1)],
                            in0=pts[jc][0:P, :],
                            scalar1=bias[0:P, :],
                            scalar2=None,
                            op0=add,
                        )

            return run

        kt_proj([0])()  # only chunk 0's K columns gate the first attention

        # ---- filler groups: emitted inside the ACT-bound attention k-loop
        #      so PE's idle slots do next-chunk proj / prev-chunk out-proj ----
        def qt_group(jc, m):
            def run():
                P = 128 if m == 0 else 64
                pt = ps.tile([128, QCH], f32, tag="st", name="qtp")
                for kd in range(ND):
                    nc.tensor.matmul(
                        pt[0:P, :],
                        wq[:, kd, 128 * m : 128 * m + P],
                        xt[:, kd, QCH * jc : QCH * (jc + 1)],
                        start=(kd == 0),
                        stop=(kd == ND - 1),
                    )
                dst = qt_a if m == 0 else qt_b
                bias = bqa if m == 0 else bqb
                nc.vector.tensor_scalar(
                    out=dst[0:P, QCH * jc : QCH * (jc + 1)],
                    in0=pt[0:P, :],
                    scalar1=bias[0:P, :],
                    scalar2=None,
                    op0=add,
                )

            return run

        def v_group(mt):
            def run():
                pv = ps.tile([128, QCH], f32, tag="st", name="pv")
                for kd in range(ND):
                    nc.tensor.matmul(
                        pv[:, 0:HS],
                        xt[:, kd, 128 * mt : 128 * (mt + 1)],
                        wv[:, kd, :],
                        start=(kd == 0),
                        stop=(kd == ND - 1),
                    )
                nc.vector.tensor_copy(vaug[:, mt, :, 0:64], pv[:, 0:HS])

            return run

        def op_group(mt, evict_act=False):
            def run():
                ob = obp.tile([128, D], f32, name="ob")
                for nh in range(2):
                    op = pm.tile([128, QCH], f32, tag="misc", name="op")
                    nc.tensor.matmul(
                        op[:, 0:384],
                        ctxt_a[:, 128 * mt : 128 * (mt + 1)],
                        wo_a[:, 384 * nh : 384 * (nh + 1)],
                        start=True,
                        stop=False,
                    )
                    nc.tensor.matmul(
                        op[:, 0:384],
                        ctxt_b[:, 128 * mt : 128 * (mt + 1)],
                        wo_b[:, 384 * nh : 384 * (nh + 1)],
                        start=False,
                        stop=True,
                    )
                    if evict_act:
                        nc.scalar.copy(
                            ob[:, 384 * nh : 384 * (nh + 1)], op[:, 0:384]
                        )
                    else:
                        nc.vector.tensor_copy(
                            ob[:, 384 * nh : 384 * (nh + 1)], op[:, 0:384]
                        )
                nc.sync.dma_start(
                    t["outp"].ap()[128 * mt : 128 * (mt + 1), :], ob[:]
                )

            return run

        def proj_groups(jc):
            return (
                [v_group(mt) for mt in range(4 * jc, 4 * (jc + 1))]
                + [qt_group(jc, 0), qt_group(jc, 1)]
            )

        # chunk 0 projections run up front
        for g in proj_groups(0):
            g()

        # ---- attention: 3 heads as parallel pipelines, ctx one k-step
        #      behind scores so PE never head-of-line blocks on ACT ----
        def head_aps(h):
            if h < 2:
                return 64 * h, qt_a, kt_a, ctxt_a
            return 0, qt_b, kt_b, ctxt_b

        scale = float(1.0 / np.sqrt(HD))

        def norm_group(jc, cxs):
            # normalize ctx^T[0:64] * broadcast(1/den), pipelined stage-major
            # across the 3 heads
            def run():
                recs, bcss = {}, {}
                for h in range(HPC):
                    rec = smp.tile([65, QCH], f32, tag="rec", name="rec")
                    nc.vector.reciprocal(rec[64:65, :], cxs[h][64:65, :])
                    recs[h] = rec
                for h in range(HPC):
                    bc = pm.tile([64, QCH], f32, tag="misc", name="bc")
                    nc.tensor.matmul(
                        bc[:], ones[64:65, :], recs[h][64:65, :],
                        start=True, stop=True,
                    )
                    bcs = smp.tile([64, QCH], f32, tag="bcs", name="bcs")
                    nc.vector.tensor_copy(bcs[:], bc[:])
                    bcss[h] = bcs
                for h in range(HPC):
                    row, _, _, dctx = head_aps(h)
                    nc.vector.scalar_tensor_tensor(
                        out=dctx[row : row + 64, QCH * jc : QCH * (jc + 1)],
                        in0=cxs[h][0:64, :],
                        scalar=1.0,
                        in1=bcss[h][:],
                        op0=mult,
                        op1=mult,
                    )

            return run

        pending = []  # deferred normalize of the previous chunk
        for jc in range(NCH):
            nkt = (QCH // KT) * (jc + 1)
            cxs, ess = {}, {}

            def scores(kt, h):
                row, qsrc, ksrc, _ = head_aps(h)
                di = kt - (QCH // KT) * jc  # diagonal-band index
                lo = 128 * di if di > 0 else 0  # valid q cols start
                st = ps.tile([128, QCH], f32, tag="st", name="st")
                nc.tensor.matmul(
                    st[:, lo:QCH],
                    ksrc[row : row + 64, KT * kt : KT * (kt + 1)],
                    qsrc[row : row + 64, QCH * jc + lo : QCH * (jc + 1)],
                    start=True,
                    stop=True,
                )
                es = esp.tile([128, QCH], bf16, name="es")
                nc.scalar.activation(es[:, lo:QCH], st[:, lo:QCH], Exp, scale=scale)
                if di >= 0:  # mask the triangular diagonal 128x128 block
                    nc.vector.tensor_mul(
                        es[:, lo : lo + KT], es[:, lo : lo + KT], tri[:]
                    )
                ess[(kt, h)] = (es, lo)

            def ctxmm(kt, h):
                es, lo = ess.pop((kt, h))
                nc.tensor.matmul(
                    cxs[h][:, lo:QCH],
                    vaug[:, kt, h, :],
                    es[:, lo:QCH],
                    start=(kt == 0),
                    stop=(kt == nkt - 1),
                )

            fillers = list(pending)  # previous chunk: normalize, then out-proj
            pending = []
            if jc == 0:
                fillers += [kt_proj([1, 2, 3])]
            if jc + 1 < NCH:
                fillers += proj_groups(jc + 1)

            for h in range(HPC):
                cxs[h] = pc.tile([65, QCH], f32, tag="cx", name="cx")
            done = 0
            for kt in range(nkt):
                for h in range(HPC):
                    scores(kt, h)
                if kt > 0:
                    for h in range(HPC):
                        ctxmm(kt - 1, h)
                want = (kt + 1) * len(fillers) // nkt
                while done < want:
                    fillers[done]()
                    done += 1
            for h in range(HPC):
                ctxmm(nkt - 1, h)

            pending = [norm_group(jc, cxs)] + [
                op_group(mt, evict_act=(jc == NCH - 1))
                for mt in range(4 * jc, 4 * (jc + 1))
            ]

        # last chunk's normalize + out-projection
        for g in pending:
            g()


def _build_program(repeat=1):
    nc = bacc.Bacc(
        "TRN2", target_bir_lowering=False, debug=False, num_devices=N_CORES
    )
    bf16 = mybir.dt.bfloat16
    f32 = mybir.dt.float32
    t = {
        "xt": nc.dram_tensor("xt", [D, S], bf16, kind="ExternalInput"),
        "wq": nc.dram_tensor("wq", [D, HS], bf16, kind="ExternalInput"),
        "wk": nc.dram_tensor("wk", [D, HS], bf16, kind="ExternalInput"),
        "wv": nc.dram_tensor("wv", [D, HS], bf16, kind="ExternalInput"),
        "wo": nc.dram_tensor("wo", [HS, D], bf16, kind="ExternalInput"),
        "bq2": nc.dram_tensor("bq2", [HS, 1], f32, kind="ExternalInput"),
        "bk2": nc.dram_tensor("bk2", [HS, 1], f32, kind="ExternalInput"),
        "tri": nc.dram_tensor("tri", [128, KT], bf16, kind="ExternalInput"),
        "outp": nc.dram_tensor("outp", [S, D], f32, kind="ExternalOutput"),
    }
    with tile.TileContext(nc) as tc:
        for _ in range(repeat):
            _emit(nc, tc, t)
    nc.compile()
    return nc


def _get_program(repeat=1):
    if repeat not in _prog_cache:
        _prog_cache[repeat] = _build_program(repeat)
    return _prog_cache[repeat]


def _in_maps(x, Wq, Wk, Wv, Wo, bq, bk):
    xts = [np.ascontiguousarray(x[b].T).astype(BF16) for b in range(B)]
    tri = (np.arange(KT)[None, :] >= np.arange(128)[:, None]).astype(BF16)
    maps = []
    for c in range(N_CORES):
        b, g = divmod(c, CPB)
        c0 = HS * g
        maps.append(
            {
                "xt": xts[b],
                "wq": np.ascontiguousarray(Wq[:, c0 : c0 + HS]).astype(BF16),
                "wk": np.ascontiguousarray(Wk[:, c0 : c0 + HS]).astype(BF16),
                "wv": np.ascontiguousarray(Wv[:, c0 : c0 + HS]).astype(BF16),
                "wo": np.ascontiguousarray(Wo[c0 : c0 + HS, :]).astype(BF16),
                "bq2": np.ascontiguousarray(bq[c0 : c0 + HS]).reshape(HS, 1).astype(F32),
                "bk2": np.ascontiguousarray(bk[c0 : c0 + HS]).reshape(HS, 1).astype(F32),
                "tri": tri,
            }
        )
    return maps


def kernel(x, Wq, bq, Wk, bk, Wv, bv, Wo, bo):
    x = np.asarray(x, F32)
    Wq = np.asarray(Wq, F32)
    Wk = np.asarray(Wk, F32)
    Wv = np.asarray(Wv, F32)
    Wo = np.asarray(Wo, F32)
    bq = np.asarray(bq, F32)
    bk = np.asarray(bk, F32)
    bv = np.asarray(bv, F32)
    bo = np.asarray(bo, F32)

    nc = _get_program()
    in_maps = _in_maps(x, Wq, Wk, Wv, Wo, bq, bk)

    res = bass_utils.run_bass_kernel_spmd(
        nc, in_maps, core_ids=list(range(N_CORES))
    )
    out = np.zeros((B, S, D), F32)
    for b in range(B):
        for g in range(CPB):
            out[b] += res.results[b * CPB + g]["outp"]
    out += (bv @ Wo + bo)[None, None, :]
    return out


# revision 41
# speedup vs baseline: 29.9961x; 1.0504x over previous
"""Causal self-attention (B=2, S=2048, D=768, H=12) on 8 trn2 NeuronCores.

Sharding: batch*heads = 24 head-instances -> 3 heads per core
(cores 0-3: batch 0, cores 4-7: batch 1; core c%4 owns heads 3*(c%4)..3*(c%4)+2).
Each core computes Q/K/V projections for its 192 output dims, causal
attention for its 3 heads, and a partial out-projection
ctx[2048,192] @ Wo[192,768].  Host sums the 4 partials per batch and adds
the bias terms (bo, and bv folded through Wo: softmax rows sum to 1, so
V+bv contributes exactly bv@Wo to every output row).

Device layouts (per core):
  xt   [768, 2048]  x[b]^T              bf16
  wq/wk/wv [768, 192] col slices        bf16
  wo   [192, 768]  row slice            bf16
  bq2/bk2 [192, 1] f32 (bias applied per-partition in the transposed
           Q^T/K^T layout during PSUM eviction)
  masks [4, 128, 512] multiplicative causal masks for diagonal-band tiles

Attention is computed in transposed score layout S^T[k,q] so that no PE
transposes are needed: S^T tile = K_tile @ Q_chunk^T (contraction over
head_dim on partitions), exp on ScalarE, then ctx^T accumulates via
V_aug^T @ expS^T where V_aug carries a ones column that yields the
softmax denominator for free.  Normalization happens per 64-row ctx^T
slab via a K=1 broadcast matmul of the reciprocal denominators.
"""

import sys

sys.path.insert(0, "/opt/trn_rl_repo")

import numpy as np
import ml_dtypes

import concourse.bass as bass
import concourse.bacc as bacc
import concourse.tile as tile
from concourse import mybir
from concourse import bass_utils

BF16 = ml_dtypes.bfloat16
F32 = np.float32

B, S, D, H, HD = 2, 2048, 768, 12, 64
N_CORES = 8
CPB = 4          # cores per batch element
HPC = 3          # heads per core
HS = HPC * HD    # 192 output dims per core
QCH = 512        # q chunk (columns per scores matmul)
KT = 128         # k tile
NCH = S // QCH   # 4
NKT = S // KT    # 16
NMT = S // 128   # 16 seq tiles
ND = D // 128    # 6 contraction tiles for projections

_prog_cache = {}


def _emit(nc, tc, t, phases=("proj", "attn", "out")):
    f32 = mybir.dt.float32
    bf16 = mybir.dt.bfloat16
    Ident = mybir.ActivationFunctionType.Identity
    Exp = mybir.ActivationFunctionType.Exp
    mult = mybir.AluOpType.mult

    import contextlib

    add = mybir.AluOpType.add

    with contextlib.ExitStack() as ctx:
        sb = ctx.enter_context(tc.tile_pool(name="sb", bufs=1))
        esp = ctx.enter_context(tc.tile_pool(name="esp", bufs=8))
        smp = ctx.enter_context(tc.tile_pool(name="smp", bufs=3))
        obp = ctx.enter_context(tc.tile_pool(name="obp", bufs=2))
        # PSUM budget (8 banks): wide [128,2,512] x2 = 4 + single 1 + cx 3
        pw = ctx.enter_context(tc.tile_pool(name="pw", bufs=2, space="PSUM"))
        pm = ctx.enter_context(tc.tile_pool(name="pm", bufs=1, space="PSUM"))
        pc = ctx.enter_context(tc.tile_pool(name="pc", bufs=3, space="PSUM"))

        # ---- loads, ordered by first use; wk/xt interleaved per kd so the
        #      K projection can start accumulating as tiles arrive ----
        bka = sb.tile([128, 1], f32)
        nc.sync.dma_start(bka[:], t["bk2"].ap()[0:128, :])
        bkb = sb.tile([64, 1], f32)
        nc.sync.dma_start(bkb[:], t["bk2"].ap()[128:HS, :])
        bqa = sb.tile([128, 1], f32)
        nc.sync.dma_start(bqa[:], t["bq2"].ap()[0:128, :])
        bqb = sb.tile([64, 1], f32)
        nc.sync.dma_start(bqb[:], t["bq2"].ap()[128:HS, :])
        tri = sb.tile([128, KT], bf16)  # tri[k, q] = 1.0 if q >= k
        nc.sync.dma_start(tri[:], t["tri"].ap())

        xt = sb.tile([128, ND, S], bf16)
        wk = sb.tile([128, ND, HS], bf16)
        xt_r = t["xt"].ap().rearrange("(n p) m -> p n m", p=128)
        wk_r = t["wk"].ap().rearrange("(n p) m -> p n m", p=128)
        for kd in range(ND):
            nc.sync.dma_start(wk[:, kd, :], wk_r[:, kd, :])
            nc.sync.dma_start(xt[:, kd, :], xt_r[:, kd, :])
        wq = sb.tile([128, ND, HS], bf16)
        nc.sync.dma_start(wq[:], t["wq"].ap().rearrange("(n p) m -> p n m", p=128))
        wv = sb.tile([128, ND, HS], bf16)
        nc.sync.dma_start(wv[:], t["wv"].ap().rearrange("(n p) m -> p n m", p=128))
        wo_a = sb.tile([128, D], bf16)
        nc.sync.dma_start(wo_a[:], t["wo"].ap()[0:128, :])
        wo_b = sb.tile([64, D], bf16)
        nc.sync.dma_start(wo_b[:], t["wo"].ap()[128:HS, :])

        ones = sb.tile([65, 64], f32)
        nc.vector.memset(ones[64:65, :], 1.0)

        qt_a = sb.tile([128, S], bf16)   # heads 0,1 of Q^T
        qt_b = sb.tile([64, S], bf16)    # head 2 of Q^T
        kt_a = sb.tile([128, S], bf16)
        kt_b = sb.tile([64, S], bf16)
        vaug = sb.tile([128, NKT, HPC, 65], bf16)
        nc.vector.memset(vaug[:, :, :, 64:65], 1.0)
        ctxt_a = sb.tile([128, S], bf16)
        ctxt_b = sb.tile([64, S], bf16)

        # ---- K^T projection helper: weights stationary per (m, kd), the
        #      given chunks' psums accumulate together ----
        def kt_proj(jcs):
            def run():
                for m in range(2):
                    P = 128 if m == 0 else 64
                    wides = [
                        pw.tile([128, 2, QCH], f32, tag="w", name="ktp")
                        for _ in range((len(jcs) + 1) // 2)
                    ]
                    slot = {
                        jc: (wides[i // 2], i % 2) for i, jc in enumerate(jcs)
                    }
                    for kd in range(ND):
                        for jc in jcs:
                            w, i = slot[jc]
                            nc.tensor.matmul(
                                w[0:P, i, :],
                                wk[:, kd, 128 * m : 128 * m + P],
                                xt[:, kd, QCH * jc : QCH * (jc + 1)],
                                start=(kd == 0),
                                stop=(kd == ND - 1),
                            )
                    dst = kt_a if m == 0 else kt_b
                    bias = bka if m == 0 else bkb
                    for jc in jcs:
                        w, i = slot[jc]
                        nc.vector.tensor_scalar(
                            out=dst[0:P, QCH * jc : QCH * (jc + 1)],
                            in0=w[0:P, i, :],
                            scalar1=bias[0:P, :],
                            scalar2=None,
                            op0=add,
                        )

            return run

        do_proj = "proj" in phases
        do_attn = "attn" in phases
        do_out = "out" in phases
        if not do_proj:
            # timing variant: skip projections, zero-fill their outputs
            nc.gpsimd.memset(qt_a[:], 0)
            nc.gpsimd.memset(qt_b[:], 0)
            nc.gpsimd.memset(kt_a[:], 0)
            nc.gpsimd.memset(kt_b[:], 0)
            nc.gpsimd.memset(vaug[:, :, :, 0:64], 0)
        if do_proj:
            kt_proj([0])()  # only chunk 0's K columns gate the first attention

        # ---- filler groups: emitted inside the ACT-bound attention k-loop
        #      so PE's idle slots do next-chunk proj / prev-chunk out-proj ----
        def qt_group(jc, m=None):
            def run():
                w = pw.tile([128, 2, QCH], f32, tag="w", name="qtp")
                for kd in range(ND):
                    nc.tensor.matmul(
                        w[:, 0, :],
                        wq[:, kd, 0:128],
                        xt[:, kd, QCH * jc : QCH * (jc + 1)],
                        start=(kd == 0),
                        stop=(kd == ND - 1),
                    )
                    nc.tensor.matmul(
                        w[0:64, 1, :],
                        wq[:, kd, 128:HS],
                        xt[:, kd, QCH * jc : QCH * (jc + 1)],
                        start=(kd == 0),
                        stop=(kd == ND - 1),
                    )
                nc.vector.tensor_scalar(
                    out=qt_a[:, QCH * jc : QCH * (jc + 1)],
                    in0=w[:, 0, :],
                    scalar1=bqa[:],
                    scalar2=None,
                    op0=add,
                )
                nc.vector.tensor_scalar(
                    out=qt_b[0:64, QCH * jc : QCH * (jc + 1)],
                    in0=w[0:64, 1, :],
                    scalar1=bqb[:],
                    scalar2=None,
                    op0=add,
                )

            return run

        def v_group(mt):
            def run():
                pv = pm.tile([128, QCH], f32, tag="s1", name="pv")
                for kd in range(ND):
                    nc.tensor.matmul(
                        pv[:, 0:HS],
                        xt[:, kd, 128 * mt : 128 * (mt + 1)],
                        wv[:, kd, :],
                        start=(kd == 0),
                        stop=(kd == ND - 1),
                    )
                nc.vector.tensor_copy(vaug[:, mt, :, 0:64], pv[:, 0:HS])

            return run

        def op_group(mt, evict_act=False):
            def run():
                ob = obp.tile([128, D], f32, name="ob")
                op = pw.tile([128, 2, QCH], f32, tag="w", name="op")
                for nh in range(2):
                    nc.tensor.matmul(
                        op[:, nh, 0:384],
                        ctxt_a[:, 128 * mt : 128 * (mt + 1)],
                        wo_a[:, 384 * nh : 384 * (nh + 1)],
                        start=True,
                        stop=False,
                    )
                for nh in range(2):
                    nc.tensor.matmul(
                        op[:, nh, 0:384],
                        ctxt_b[:, 128 * mt : 128 * (mt + 1)],
                        wo_b[:, 384 * nh : 384 * (nh + 1)],
                        start=False,
                        stop=True,
                    )
                ob2 = ob[:].rearrange("p (n m) -> p n m", n=2)
                if evict_act:
                    nc.scalar.copy(ob2, op[:, :, 0:384])
                else:
                    nc.vector.tensor_copy(ob2, op[:, :, 0:384])
                nc.sync.dma_start(
                    t["outp"].ap()[128 * mt : 128 * (mt + 1), :], ob[:]
                )

            return run

        def proj_groups(jc):
            return [v_group(mt) for mt in range(4 * jc, 4 * (jc + 1))] + [
                qt_group(jc)
            ]

        # chunk 0 projections run up front
        if do_proj:
            for g in proj_groups(0):
                g()

        # ---- attention: 3 heads as parallel pipelines, ctx one k-step
        #      behind scores so PE never head-of-line blocks on ACT ----
        def head_aps(h):
            if h < 2:
                return 64 * h, qt_a, kt_a, ctxt_a
            return 0, qt_b, kt_b, ctxt_b

        scale = float(1.0 / np.sqrt(HD))

        def norm_head(jc, cxs, h):
            # normalize ctx^T[0:64] * broadcast(1/den) for one head
            def run():
                rec = smp.tile([65, QCH], f32, tag="rec", name="rec")
                nc.vector.reciprocal(rec[64:65, :], cxs[h][64:65, :])
                bc = pm.tile([64, QCH], f32, tag="s1", name="bc")
                nc.tensor.matmul(
                    bc[:], ones[64:65, :], rec[64:65, :], start=True, stop=True
                )
                bcs = smp.tile([64, QCH], f32, tag="bcs", name="bcs")
                nc.vector.tensor_copy(bcs[:], bc[:])
                row, _, _, dctx = head_aps(h)
                nc.vector.scalar_tensor_tensor(
                    out=dctx[row : row + 64, QCH * jc : QCH * (jc + 1)],
                    in0=cxs[h][0:64, :],
                    scalar=1.0,
                    in1=bcs[:],
                    op0=mult,
                    op1=mult,
                )

            return run

        def norm_group(jc, cxs):
            heads = [norm_head(jc, cxs, h) for h in range(HPC)]

            def run():
                for g in heads:
                    g()

            return run

        if do_proj and not do_attn:
            kt_proj([1, 2, 3])()
            for jc2 in range(1, NCH):
                for g in proj_groups(jc2):
                    g()
        pending = []  # deferred normalize of the previous chunk
        for jc in range(NCH) if do_attn else []:
            nkt = (QCH // KT) * (jc + 1)
            npair = nkt // 2
            cxs, ess = {}, {}

            def scores_pair(p, h):
                row, qsrc, ksrc, _ = head_aps(h)
                w = pw.tile([128, 2, QCH], f32, tag="w", name="w")
                es = esp.tile([128, 2, QCH], bf16, name="es")
                los = []
                for i in range(2):
                    kt = 2 * p + i
                    di = kt - (QCH // KT) * jc  # diagonal-band index
                    lo = 128 * di if di > 0 else 0  # valid q cols start
                    nc.tensor.matmul(
                        w[:, i, lo:QCH],
                        ksrc[row : row + 64, KT * kt : KT * (kt + 1)],
                        qsrc[row : row + 64, QCH * jc + lo : QCH * (jc + 1)],
                        start=True,
                        stop=True,
                    )
                    los.append((kt, di, lo))
                if los[0][1] >= 0 or los[1][1] >= 0:
                    # diagonal-band pair: separate column-sliced exps
                    for i, (kt, di, lo) in enumerate(los):
                        nc.scalar.activation(
                            es[:, i, lo:QCH], w[:, i, lo:QCH], Exp, scale=scale
                        )
                else:
                    # off-diagonal: one exp spanning both k-tiles
                    nc.scalar.activation(es[:, :, :], w[:, :, :], Exp, scale=scale)
                for i, (kt, di, lo) in enumerate(los):
                    if di >= 0:  # mask the triangular diagonal 128x128 block
                        nc.vector.tensor_mul(
                            es[:, i, lo : lo + KT], es[:, i, lo : lo + KT], tri[:]
                        )
                ess[(p, h)] = (es, los)

            def ctx_pair(p, h):
                es, los = ess.pop((p, h))
                for i, (kt, di, lo) in enumerate(los):
                    nc.tensor.matmul(
                        cxs[h][:, lo:QCH],
                        vaug[:, kt, h, :],
                        es[:, i, lo:QCH],
                        start=(kt == 0),
                        stop=(kt == nkt - 1),
                    )

            fillers = list(pending)  # previous chunk: normalize, then out-proj
            pending = []
            if do_proj:
                if jc == 0:
                    fillers += [kt_proj([1, 2, 3])]
                if jc + 1 < NCH:
                    fillers += proj_groups(jc + 1)

            for h in range(HPC):
                cxs[h] = pc.tile([65, QCH], f32, tag="cx", name="cx")
            done = 0
            for p in range(npair):
                for h in range(HPC):
                    scores_pair(p, h)
                if p > 0:
                    for h in range(HPC):
                        ctx_pair(p - 1, h)
                want = (p + 1) * len(fillers) // npair
                while done < want:
                    fillers[done]()
                    done += 1
            for h in range(HPC):
                ctx_pair(npair - 1, h)

            pending = [norm_head(jc, cxs, h) for h in range(HPC)]
            if do_out:
                pending += [
                    op_group(mt, evict_act=(jc == NCH - 1))
                    for mt in range(4 * jc, 4 * (jc + 1))
                ]

        # last chunk's normalize + out-projection
        for g in pending:
            g()

        if not (do_proj and do_attn and do_out):
            # timing variants: consume live tensors so DCE keeps the work
            sinks = []
            if do_proj:
                sinks += [qt_a[:, 0:D], kt_a[:, 0:D], vaug[:, 0:4, :, 0:64]]
            if do_attn:
                sinks += [ctxt_a[:, 0:D], ctxt_b[0:64, 0:D]]
            for i, src in enumerate(sinks):
                snk = obp.tile([128, D], f32, name="snk")
                P = src.partition_size()
                nc.vector.tensor_copy(snk[0:P, :], src)
                nc.sync.dma_start(
                    t["outp"].ap()[128 * i : 128 * i + P, :], snk[0:P, :]
                )


def _build_program(repeat=1, phases=("proj", "attn", "out")):
    nc = bacc.Bacc(
        "TRN2", target_bir_lowering=False, debug=False, num_devices=N_CORES
    )
    bf16 = mybir.dt.bfloat16
    f32 = mybir.dt.float32
    t = {
        "xt": nc.dram_tensor("xt", [D, S], bf16, kind="ExternalInput"),
        "wq": nc.dram_tensor("wq", [D, HS], bf16, kind="ExternalInput"),
        "wk": nc.dram_tensor("wk", [D, HS], bf16, kind="ExternalInput"),
        "wv": nc.dram_tensor("wv", [D, HS], bf16, kind="ExternalInput"),
        "wo": nc.dram_tensor("wo", [HS, D], bf16, kind="ExternalInput"),
        "bq2": nc.dram_tensor("bq2", [HS, 1], f32, kind="ExternalInput"),
        "bk2": nc.dram_tensor("bk2", [HS, 1], f32, kind="ExternalInput"),
        "tri": nc.dram_tensor("tri", [128, KT], bf16, kind="ExternalInput"),
        "outp": nc.dram_tensor("outp", [S, D], f32, kind="ExternalOutput"),
    }
    with tile.TileContext(nc) as tc:
        for _ in range(repeat):
            _emit(nc, tc, t, phases)
    nc.compile()
    return nc


def _get_program(repeat=1, phases=("proj", "attn", "out")):
    key = (repeat, tuple(phases))
    if key not in _prog_cache:
        _prog_cache[key] = _build_program(repeat, phases)
    return _prog_cache[key]


def _in_maps(x, Wq, Wk, Wv, Wo, bq, bk):
    xts = [np.ascontiguousarray(x[b].T).astype(BF16) for b in range(B)]
    tri = (np.arange(KT)[None, :] >= np.arange(128)[:, None]).astype(BF16)
    maps = []
    for c in range(N_CORES):
        b, g = divmod(c, CPB)
        c0 = HS * g
        maps.append(
            {
                "xt": xts[b],
                "wq": np.ascontiguousarray(Wq[:, c0 : c0 + HS]).astype(BF16),
                "wk": np.ascontiguousarray(Wk[:, c0 : c0 + HS]).astype(BF16),
                "wv": np.ascontiguousarray(Wv[:, c0 : c0 + HS]).astype(BF16),
                "wo": np.ascontiguousarray(Wo[c0 : c0 + HS, :]).astype(BF16),
                "bq2": np.ascontiguousarray(bq[c0 : c0 + HS]).reshape(HS, 1).astype(F32),
                "bk2": np.ascontiguousarray(bk[c0 : c0 + HS]).reshape(HS, 1).astype(F32),
                "tri": tri,
            }
        )
    return maps


def kernel(x, Wq, bq, Wk, bk, Wv, bv, Wo, bo):
    x = np.asarray(x, F32)
    Wq = np.asarray(Wq, F32)
    Wk = np.asarray(Wk, F32)
    Wv = np.asarray(Wv, F32)
    Wo = np.asarray(Wo, F32)
    bq = np.asarray(bq, F32)
    bk = np.asarray(bk, F32)
    bv = np.asarray(bv, F32)
    bo = np.asarray(bo, F32)

    nc = _get_program()
    in_maps = _in_maps(x, Wq, Wk, Wv, Wo, bq, bk)

    res = bass_utils.run_bass_kernel_spmd(
        nc, in_maps, core_ids=list(range(N_CORES))
    )
    out = np.zeros((B, S, D), F32)
    for b in range(B):
        for g in range(CPB):
            out[b] += res.results[b * CPB + g]["outp"]
    out += (bv @ Wo + bo)[None, None, :]
    return out


# revision 45
# speedup vs baseline: 30.4897x; 1.0165x over previous
"""Causal self-attention (B=2, S=2048, D=768, H=12) on 8 trn2 NeuronCores.

Sharding: batch*heads = 24 head-instances -> 3 heads per core
(cores 0-3: batch 0, cores 4-7: batch 1; core c%4 owns heads 3*(c%4)..3*(c%4)+2).
Each core computes Q/K/V projections for its 192 output dims, causal
attention for its 3 heads, and a partial out-projection
ctx[2048,192] @ Wo[192,768].  Host sums the 4 partials per batch and adds
the bias terms (bo, and bv folded through Wo: softmax rows sum to 1, so
V+bv contributes exactly bv@Wo to every output row).

Device layouts (per core):
  xt   [768, 2048]  x[b]^T              bf16
  wq/wk/wv [768, 192] col slices        bf16
  wo   [192, 768]  row slice            bf16
  bq2/bk2 [192, 1] f32 (bias applied per-partition in the transposed
           Q^T/K^T layout during PSUM eviction)
  masks [4, 128, 512] multiplicative causal masks for diagonal-band tiles

Attention is computed in transposed score layout S^T[k,q] so that no PE
transposes are needed: S^T tile = K_tile @ Q_chunk^T (contraction over
head_dim on partitions), exp on ScalarE, then ctx^T accumulates via
V_aug^T @ expS^T where V_aug carries a ones column that yields the
softmax denominator for free.  Normalization happens per 64-row ctx^T
slab via a K=1 broadcast matmul of the reciprocal denominators.
"""

import sys

sys.path.insert(0, "/opt/trn_rl_repo")

import numpy as np
import ml_dtypes

import concourse.bass as bass
import concourse.bacc as bacc
import concourse.tile as tile
from concourse import mybir
from concourse import bass_utils

BF16 = ml_dtypes.bfloat16
F32 = np.float32

B, S, D, H, HD = 2, 2048, 768, 12, 64
N_CORES = 8
CPB = 4          # cores per batch element
HPC = 3          # heads per core
HS = HPC * HD    # 192 output dims per core
QCH = 512        # q chunk (columns per scores matmul)
KT = 128         # k tile
NCH = S // QCH   # 4
NKT = S // KT    # 16
NMT = S // 128   # 16 seq tiles
ND = D // 128    # 6 contraction tiles for projections

_prog_cache = {}


def _emit(nc, tc, t, phases=("proj", "attn", "out")):
    f32 = mybir.dt.float32
    bf16 = mybir.dt.bfloat16
    Exp = mybir.ActivationFunctionType.Exp
    mult = mybir.AluOpType.mult
    add = mybir.AluOpType.add

    import contextlib

    with contextlib.ExitStack() as ctx:
        sb = ctx.enter_context(tc.tile_pool(name="sb", bufs=1))
        esp = ctx.enter_context(tc.tile_pool(name="esp", bufs=8))
        smp = ctx.enter_context(tc.tile_pool(name="smp", bufs=3))
        obp = ctx.enter_context(tc.tile_pool(name="obp", bufs=2))
        # PSUM budget (8 banks): wide [128,2,512] x2 = 4 + single 1 + cx 3
        pw = ctx.enter_context(tc.tile_pool(name="pw", bufs=2, space="PSUM"))
        pm = ctx.enter_context(tc.tile_pool(name="pm", bufs=1, space="PSUM"))
        pc = ctx.enter_context(tc.tile_pool(name="pc", bufs=3, space="PSUM"))

        # ---- loads, ordered by first use; wk/xt interleaved per kd so the
        #      K projection can start accumulating as tiles arrive ----
        bka = sb.tile([128, 1], f32)
        nc.sync.dma_start(bka[:], t["bk2"].ap()[0:128, :])
        bkb = sb.tile([64, 1], f32)
        nc.sync.dma_start(bkb[:], t["bk2"].ap()[128:HS, :])
        bqa = sb.tile([128, 1], f32)
        nc.sync.dma_start(bqa[:], t["bq2"].ap()[0:128, :])
        bqb = sb.tile([64, 1], f32)
        nc.sync.dma_start(bqb[:], t["bq2"].ap()[128:HS, :])
        tri = sb.tile([128, KT], bf16)  # tri[k, q] = 1.0 if q >= k
        nc.sync.dma_start(tri[:], t["tri"].ap())

        xt = sb.tile([128, ND, S], bf16)
        wk = sb.tile([128, ND, HS], bf16)
        xt_r = t["xt"].ap().rearrange("(n p) m -> p n m", p=128)
        wk_r = t["wk"].ap().rearrange("(n p) m -> p n m", p=128)
        for kd in range(ND):
            nc.sync.dma_start(wk[:, kd, :], wk_r[:, kd, :])
            nc.sync.dma_start(xt[:, kd, :], xt_r[:, kd, :])
        wq = sb.tile([128, ND, HS], bf16)
        nc.sync.dma_start(wq[:], t["wq"].ap().rearrange("(n p) m -> p n m", p=128))
        wv = sb.tile([128, ND, HS], bf16)
        nc.sync.dma_start(wv[:], t["wv"].ap().rearrange("(n p) m -> p n m", p=128))
        wo_a = sb.tile([128, D], bf16)
        nc.sync.dma_start(wo_a[:], t["wo"].ap()[0:128, :])
        wo_b = sb.tile([64, D], bf16)
        nc.sync.dma_start(wo_b[:], t["wo"].ap()[128:HS, :])

        qt_a = sb.tile([128, S], bf16)   # heads 0,1 of Q^T
        qt_b = sb.tile([64, S], bf16)    # head 2 of Q^T
        kt_a = sb.tile([128, S], bf16)
        kt_b = sb.tile([64, S], bf16)
        # per (k-tile, head): [V | 64 ones columns] -> the ctx matmul yields
        # ctx^T in rows 0-63 and the softmax denominator replicated across
        # rows 64-127 (already broadcast for the normalize multiply)
        vaug = sb.tile([128, NKT, HPC, 128], bf16)
        nc.vector.memset(vaug[:, :, :, 64:128], 1.0)
        ctxt_a = sb.tile([128, S], bf16)
        ctxt_b = sb.tile([64, S], bf16)

        # ---- K^T projection helper: weights stationary per (m, kd), the
        #      given chunks' psums accumulate together ----
        def kt_proj(jcs):
            def run():
                for m in range(2):
                    P = 128 if m == 0 else 64
                    wides = [
                        pw.tile([128, 2, QCH], f32, tag="w", name="ktp")
                        for _ in range((len(jcs) + 1) // 2)
                    ]
                    slot = {
                        jc: (wides[i // 2], i % 2) for i, jc in enumerate(jcs)
                    }
                    for kd in range(ND):
                        for jc in jcs:
                            w, i = slot[jc]
                            nc.tensor.matmul(
                                w[0:P, i, :],
                                wk[:, kd, 128 * m : 128 * m + P],
                                xt[:, kd, QCH * jc : QCH * (jc + 1)],
                                start=(kd == 0),
                                stop=(kd == ND - 1),
                            )
                    dst = kt_a if m == 0 else kt_b
                    bias = bka if m == 0 else bkb
                    for jc in jcs:
                        w, i = slot[jc]
                        nc.vector.tensor_scalar(
                            out=dst[0:P, QCH * jc : QCH * (jc + 1)],
                            in0=w[0:P, i, :],
                            scalar1=bias[0:P, :],
                            scalar2=None,
                            op0=add,
                        )

            return run

        do_proj = "proj" in phases
        do_attn = "attn" in phases
        do_out = "out" in phases
        if not do_proj:
            # timing variant: skip projections, zero-fill their outputs
            nc.gpsimd.memset(qt_a[:], 0)
            nc.gpsimd.memset(qt_b[:], 0)
            nc.gpsimd.memset(kt_a[:], 0)
            nc.gpsimd.memset(kt_b[:], 0)
            nc.gpsimd.memset(vaug[:, :, :, 0:64], 0)
        if do_proj:
            kt_proj([0])()  # only chunk 0's K columns gate the first attention

        # ---- filler groups: emitted inside the ACT-bound attention k-loop
        #      so PE's idle slots do next-chunk proj / prev-chunk out-proj ----
        def qt_group(jc, m=None):
            def run():
                w = pw.tile([128, 2, QCH], f32, tag="w", name="qtp")
                for kd in range(ND):
                    nc.tensor.matmul(
                        w[:, 0, :],
                        wq[:, kd, 0:128],
                        xt[:, kd, QCH * jc : QCH * (jc + 1)],
                        start=(kd == 0),
                        stop=(kd == ND - 1),
                    )
                    nc.tensor.matmul(
                        w[0:64, 1, :],
                        wq[:, kd, 128:HS],
                        xt[:, kd, QCH * jc : QCH * (jc + 1)],
                        start=(kd == 0),
                        stop=(kd == ND - 1),
                    )
                nc.vector.tensor_scalar(
                    out=qt_a[:, QCH * jc : QCH * (jc + 1)],
                    in0=w[:, 0, :],
                    scalar1=bqa[:],
                    scalar2=None,
                    op0=add,
                )
                nc.vector.tensor_scalar(
                    out=qt_b[0:64, QCH * jc : QCH * (jc + 1)],
                    in0=w[0:64, 1, :],
                    scalar1=bqb[:],
                    scalar2=None,
                    op0=add,
                )

            return run

        def v_group(mt):
            def run():
                pv = pm.tile([128, QCH], f32, tag="s1", name="pv")
                for kd in range(ND):
                    nc.tensor.matmul(
                        pv[:, 0:HS],
                        xt[:, kd, 128 * mt : 128 * (mt + 1)],
                        wv[:, kd, :],
                        start=(kd == 0),
                        stop=(kd == ND - 1),
                    )
                nc.vector.tensor_copy(vaug[:, mt, :, 0:64], pv[:, 0:HS])

            return run

        def op_group(mt, evict_act=False):
            def run():
                ob = obp.tile([128, D], f32, name="ob")
                op = pw.tile([128, 2, QCH], f32, tag="w", name="op")
                for nh in range(2):
                    nc.tensor.matmul(
                        op[:, nh, 0:384],
                        ctxt_a[:, 128 * mt : 128 * (mt + 1)],
                        wo_a[:, 384 * nh : 384 * (nh + 1)],
                        start=True,
                        stop=False,
                    )
                for nh in range(2):
                    nc.tensor.matmul(
                        op[:, nh, 0:384],
                        ctxt_b[:, 128 * mt : 128 * (mt + 1)],
                        wo_b[:, 384 * nh : 384 * (nh + 1)],
                        start=False,
                        stop=True,
                    )
                ob2 = ob[:].rearrange("p (n m) -> p n m", n=2)
                if evict_act:
                    nc.scalar.copy(ob2, op[:, :, 0:384])
                else:
                    nc.vector.tensor_copy(ob2, op[:, :, 0:384])
                nc.sync.dma_start(
                    t["outp"].ap()[128 * mt : 128 * (mt + 1), :], ob[:]
                )

            return run

        def proj_groups(jc):
            return [v_group(mt) for mt in range(4 * jc, 4 * (jc + 1))] + [
                qt_group(jc)
            ]

        # chunk 0 projections run up front
        if do_proj:
            for g in proj_groups(0):
                g()

        # ---- attention: 3 heads as parallel pipelines, ctx one k-step
        #      behind scores so PE never head-of-line blocks on ACT ----
        def head_aps(h):
            if h < 2:
                return 64 * h, qt_a, kt_a, ctxt_a
            return 0, qt_b, kt_b, ctxt_b

        scale = float(1.0 / np.sqrt(HD))

        def norm_head(jc, cxs, h):
            # rows 64-127 of cx hold the denominator replicated: the chain is
            # just reciprocal -> multiply (no broadcast matmul needed)
            def run():
                rec = smp.tile([128, QCH], f32, tag="rec", name="rec")
                nc.vector.reciprocal(rec[64:128, :], cxs[h][64:128, :])
                row, _, _, dctx = head_aps(h)
                nc.vector.scalar_tensor_tensor(
                    out=dctx[row : row + 64, QCH * jc : QCH * (jc + 1)],
                    in0=cxs[h][0:64, :],
                    scalar=1.0,
                    in1=rec[64:128, :],
                    op0=mult,
                    op1=mult,
                )

            return run

        if do_proj and not do_attn:
            kt_proj([1, 2, 3])()
            for jc2 in range(1, NCH):
                for g in proj_groups(jc2):
                    g()
        pending = []  # deferred normalize of the previous chunk
        for jc in range(NCH) if do_attn else []:
            nkt = (QCH // KT) * (jc + 1)
            npair = nkt // 2
            cxs, ess = {}, {}

            def scores_pair(p, h):
                row, qsrc, ksrc, _ = head_aps(h)
                w = pw.tile([128, 2, QCH], f32, tag="w", name="w")
                es = esp.tile([128, 2, QCH], bf16, name="es")
                los = []
                for i in range(2):
                    kt = 2 * p + i
                    di = kt - (QCH // KT) * jc  # diagonal-band index
                    lo = 128 * di if di > 0 else 0  # valid q cols start
                    nc.tensor.matmul(
                        w[:, i, lo:QCH],
                        ksrc[row : row + 64, KT * kt : KT * (kt + 1)],
                        qsrc[row : row + 64, QCH * jc + lo : QCH * (jc + 1)],
                        start=True,
                        stop=True,
                    )
                    los.append((kt, di, lo))
                if los[0][1] >= 0 or los[1][1] >= 0:
                    # diagonal-band pair: separate column-sliced exps
                    for i, (kt, di, lo) in enumerate(los):
                        nc.scalar.activation(
                            es[:, i, lo:QCH], w[:, i, lo:QCH], Exp, scale=scale
                        )
                else:
                    # off-diagonal: one exp spanning both k-tiles
                    nc.scalar.activation(es[:, :, :], w[:, :, :], Exp, scale=scale)
                for i, (kt, di, lo) in enumerate(los):
                    if di >= 0:  # mask the triangular diagonal 128x128 block
                        nc.vector.tensor_mul(
                            es[:, i, lo : lo + KT], es[:, i, lo : lo + KT], tri[:]
                        )
                ess[(p, h)] = (es, los)

            def ctx_pair(p, h):
                es, los = ess.pop((p, h))
                for i, (kt, di, lo) in enumerate(los):
                    nc.tensor.matmul(
                        cxs[h][:, lo:QCH],
                        vaug[:, kt, h, :],
                        es[:, i, lo:QCH],
                        start=(kt == 0),
                        stop=(kt == nkt - 1),
                    )

            fillers = list(pending)  # previous chunk: normalize, then out-proj
            pending = []
            if do_proj:
                if jc == 0:
                    fillers += [kt_proj([1, 2, 3])]
                if jc + 1 < NCH:
                    fillers += proj_groups(jc + 1)

            for h in range(HPC):
                cxs[h] = pc.tile([128, QCH], f32, tag="cx", name="cx")
            done = 0
            for p in range(npair):
                for h in range(HPC):
                    scores_pair(p, h)
                if p > 0:
                    for h in range(HPC):
                        ctx_pair(p - 1, h)
                want = (p + 1) * len(fillers) // npair
                while done < want:
                    fillers[done]()
                    done += 1
            for h in range(HPC):
                ctx_pair(npair - 1, h)

            pending = [norm_head(jc, cxs, h) for h in range(HPC)]
            if do_out:
                pending += [
                    op_group(mt, evict_act=(jc == NCH - 1))
                    for mt in range(4 * jc, 4 * (jc + 1))
                ]

        # last chunk's normalize + out-projection
        for g in pending:
            g()

        if not (do_proj and do_attn and do_out):
            # timing variants: consume live tensors so DCE keeps the work
            sinks = []
            if do_proj:
                sinks += [qt_a[:, 0:D], kt_a[:, 0:D], vaug[:, 0:4, :, 0:64]]
            if do_attn:
                sinks += [ctxt_a[:, 0:D], ctxt_b[0:64, 0:D]]
            for i, src in enumerate(sinks):
                snk = obp.tile([128, D], f32, name="snk")
                P = src.partition_size()
                nc.vector.tensor_copy(snk[0:P, :], src)
                nc.sync.dma_start(
                    t["outp"].ap()[128 * i : 128 * i + P, :], snk[0:P, :]
                )


def _build_program(repeat=1, phases=("proj", "attn", "out")):
    nc = bacc.Bacc(
        "TRN2", target_bir_lowering=False, debug=False, num_devices=N_CORES
    )
    bf16 = mybir.dt.bfloat16
    f32 = mybir.dt.float32
    t = {
        "xt": nc.dram_tensor("xt", [D, S], bf16, kind="ExternalInput"),
        "wq": nc.dram_tensor("wq", [D, HS], bf16, kind="ExternalInput"),
        "wk": nc.dram_tensor("wk", [D, HS], bf16, kind="ExternalInput"),
        "wv": nc.dram_tensor("wv", [D, HS], bf16, kind="ExternalInput"),
        "wo": nc.dram_tensor("wo", [HS, D], bf16, kind="ExternalInput"),
        "bq2": nc.dram_tensor("bq2", [HS, 1], f32, kind="ExternalInput"),
        "bk2": nc.dram_tensor("bk2", [HS, 1], f32, kind="ExternalInput"),
        "tri": nc.dram_tensor("tri", [128, KT], bf16, kind="ExternalInput"),
        "outp": nc.dram_tensor("outp", [S, D], f32, kind="ExternalOutput"),
    }
    with tile.TileContext(nc) as tc:
        for _ in range(repeat):
            _emit(nc, tc, t, phases)
    nc.compile()
    return nc


def _get_program(repeat=1, phases=("proj", "attn", "out")):
    key = (repeat, tuple(phases))
    if key not in _prog_cache:
        _prog_cache[key] = _build_program(repeat, phases)
    return _prog_cache[key]


def _in_maps(x, Wq, Wk, Wv, Wo, bq, bk):
    xts = [np.ascontiguousarray(x[b].T).astype(BF16) for b in range(B)]
    tri = (np.arange(KT)[None, :] >= np.arange(128)[:, None]).astype(BF16)
    maps = []
    for c in range(N_CORES):
        b, g = divmod(c, CPB)
        c0 = HS * g
        maps.append(
            {
                "xt": xts[b],
                "wq": np.ascontiguousarray(Wq[:, c0 : c0 + HS]).astype(BF16),
                "wk": np.ascontiguousarray(Wk[:, c0 : c0 + HS]).astype(BF16),
                "wv": np.ascontiguousarray(Wv[:, c0 : c0 + HS]).astype(BF16),
                "wo": np.ascontiguousarray(Wo[c0 : c0 + HS, :]).astype(BF16),
                "bq2": np.ascontiguousarray(bq[c0 : c0 + HS]).reshape(HS, 1).astype(F32),
                "bk2": np.ascontiguousarray(bk[c0 : c0 + HS]).reshape(HS, 1).astype(F32),
                "tri": tri,
            }
        )
    return maps


def kernel(x, Wq, bq, Wk, bk, Wv, bv, Wo, bo):
    x = np.asarray(x, F32)
    Wq = np.asarray(Wq, F32)
    Wk = np.asarray(Wk, F32)
    Wv = np.asarray(Wv, F32)
    Wo = np.asarray(Wo, F32)
    bq = np.asarray(bq, F32)
    bk = np.asarray(bk, F32)
    bv = np.asarray(bv, F32)
    bo = np.asarray(bo, F32)

    nc = _get_program()
    in_maps = _in_maps(x, Wq, Wk, Wv, Wo, bq, bk)

    res = bass_utils.run_bass_kernel_spmd(
        nc, in_maps, core_ids=list(range(N_CORES))
    )
    out = np.zeros((B, S, D), F32)
    for b in range(B):
        for g in range(CPB):
            out[b] += res.results[b * CPB + g]["outp"]
    out += (bv @ Wo + bo)[None, None, :]
    return out


# revision 48
# speedup vs baseline: 31.9824x; 1.0490x over previous
"""Causal self-attention (B=2, S=2048, D=768, H=12) on 8 trn2 NeuronCores.

Sharding: batch*heads = 24 head-instances -> 3 heads per core
(cores 0-3: batch 0, cores 4-7: batch 1; core c%4 owns heads 3*(c%4)..3*(c%4)+2).
Each core computes Q/K/V projections for its 192 output dims, causal
attention for its 3 heads, and a partial out-projection
ctx[2048,192] @ Wo[192,768].  Host sums the 4 partials per batch and adds
the bias terms (bo, and bv folded through Wo: softmax rows sum to 1, so
V+bv contributes exactly bv@Wo to every output row).

Device layouts (per core):
  xt   [768, 2048]  x[b]^T              bf16
  wq/wk/wv [768, 192] col slices        bf16
  wo   [192, 768]  row slice            bf16
  bq2/bk2 [192, 1] f32 (bias applied per-partition in the transposed
           Q^T/K^T layout during PSUM eviction)
  masks [4, 128, 512] multiplicative causal masks for diagonal-band tiles

Attention is computed in transposed score layout S^T[k,q] so that no PE
transposes are needed: S^T tile = K_tile @ Q_chunk^T (contraction over
head_dim on partitions), exp on ScalarE, then ctx^T accumulates via
V_aug^T @ expS^T where V_aug carries a ones column that yields the
softmax denominator for free.  Normalization happens per 64-row ctx^T
slab via a K=1 broadcast matmul of the reciprocal denominators.
"""

import sys

sys.path.insert(0, "/opt/trn_rl_repo")

import numpy as np
import ml_dtypes

import concourse.bass as bass
import concourse.bacc as bacc
import concourse.tile as tile
from concourse import mybir
from concourse import bass_utils

BF16 = ml_dtypes.bfloat16
F32 = np.float32

B, S, D, H, HD = 2, 2048, 768, 12, 64
N_CORES = 8
CPB = 4          # cores per batch element
HPC = 3          # heads per core
HS = HPC * HD    # 192 output dims per core
QCH = 512        # q chunk (columns per scores matmul)
KT = 128         # k tile
NCH = S // QCH   # 4
NKT = S // KT    # 16
NMT = S // 128   # 16 seq tiles
ND = D // 128    # 6 contraction tiles for projections

_prog_cache = {}


def _emit(nc, tc, t, phases=("proj", "attn", "out")):
    f32 = mybir.dt.float32
    bf16 = mybir.dt.bfloat16
    Exp = mybir.ActivationFunctionType.Exp
    mult = mybir.AluOpType.mult
    add = mybir.AluOpType.add

    import contextlib

    with contextlib.ExitStack() as ctx:
        sb = ctx.enter_context(tc.tile_pool(name="sb", bufs=1))
        esp = ctx.enter_context(tc.tile_pool(name="esp", bufs=8))
        smp = ctx.enter_context(tc.tile_pool(name="smp", bufs=3))
        obp = ctx.enter_context(tc.tile_pool(name="obp", bufs=2))
        # PSUM budget (8 banks): wide [128,2,512] x2 = 4 + single 1 + cx 3
        pw = ctx.enter_context(tc.tile_pool(name="pw", bufs=2, space="PSUM"))
        pm = ctx.enter_context(tc.tile_pool(name="pm", bufs=1, space="PSUM"))
        pc = ctx.enter_context(tc.tile_pool(name="pc", bufs=3, space="PSUM"))

        # ---- loads, ordered by first use; wk/xt interleaved per kd so the
        #      K projection can start accumulating as tiles arrive ----
        bka = sb.tile([128, 1], f32)
        nc.sync.dma_start(bka[:], t["bk2"].ap()[0:128, :])
        bkb = sb.tile([64, 1], f32)
        nc.sync.dma_start(bkb[:], t["bk2"].ap()[128:HS, :])
        bqa = sb.tile([128, 1], f32)
        nc.sync.dma_start(bqa[:], t["bq2"].ap()[0:128, :])
        bqb = sb.tile([64, 1], f32)
        nc.sync.dma_start(bqb[:], t["bq2"].ap()[128:HS, :])
        tri = sb.tile([128, KT], bf16)  # tri[k, q] = 1.0 if q >= k
        nc.sync.dma_start(tri[:], t["tri"].ap())

        xt = sb.tile([128, ND, S], bf16)
        wk = sb.tile([128, ND, HS], bf16)
        xt_r = t["xt"].ap().rearrange("(n p) m -> p n m", p=128)
        wk_r = t["wk"].ap().rearrange("(n p) m -> p n m", p=128)
        for kd in range(ND):
            nc.sync.dma_start(wk[:, kd, :], wk_r[:, kd, :])
            nc.sync.dma_start(xt[:, kd, :], xt_r[:, kd, :])
        wq = sb.tile([128, ND, HS], bf16)
        nc.sync.dma_start(wq[:], t["wq"].ap().rearrange("(n p) m -> p n m", p=128))
        wv = sb.tile([128, ND, HS], bf16)
        nc.sync.dma_start(wv[:], t["wv"].ap().rearrange("(n p) m -> p n m", p=128))
        wo_a = sb.tile([128, D], bf16)
        nc.sync.dma_start(wo_a[:], t["wo"].ap()[0:128, :])
        wo_b = sb.tile([64, D], bf16)
        nc.sync.dma_start(wo_b[:], t["wo"].ap()[128:HS, :])

        qt_a = sb.tile([128, S], bf16)   # heads 0,1 of Q^T
        qt_b = sb.tile([64, S], bf16)    # head 2 of Q^T
        kt_a = sb.tile([128, S], bf16)
        kt_b = sb.tile([64, S], bf16)
        # per (k-tile, head): [V | 64 ones columns] -> the ctx matmul yields
        # ctx^T in rows 0-63 and the softmax denominator replicated across
        # rows 64-127 (already broadcast for the normalize multiply)
        vaug = sb.tile([128, NKT, HPC, 128], bf16)
        nc.vector.memset(vaug[:, :, :, 64:128], 1.0)
        ctxt_a = sb.tile([128, S], bf16)
        ctxt_b = sb.tile([64, S], bf16)

        # ---- K^T projection helper: weights stationary per (m, kd), the
        #      given chunks' psums accumulate together ----
        def kt_proj(jcs):
            def run():
                for m in range(2):
                    P = 128 if m == 0 else 64
                    wides = [
                        pw.tile([128, 2, QCH], f32, tag="w", name="ktp")
                        for _ in range((len(jcs) + 1) // 2)
                    ]
                    slot = {
                        jc: (wides[i // 2], i % 2) for i, jc in enumerate(jcs)
                    }
                    for kd in range(ND):
                        for jc in jcs:
                            w, i = slot[jc]
                            nc.tensor.matmul(
                                w[0:P, i, :],
                                wk[:, kd, 128 * m : 128 * m + P],
                                xt[:, kd, QCH * jc : QCH * (jc + 1)],
                                start=(kd == 0),
                                stop=(kd == ND - 1),
                            )
                    dst = kt_a if m == 0 else kt_b
                    bias = bka if m == 0 else bkb
                    for jc in jcs:
                        w, i = slot[jc]
                        nc.vector.tensor_scalar(
                            out=dst[0:P, QCH * jc : QCH * (jc + 1)],
                            in0=w[0:P, i, :],
                            scalar1=bias[0:P, :],
                            scalar2=None,
                            op0=add,
                        )

            return run

        do_proj = "proj" in phases
        do_attn = "attn" in phases
        do_out = "out" in phases
        if not do_proj:
            # timing variant: skip projections, zero-fill their outputs
            nc.gpsimd.memset(qt_a[:], 0)
            nc.gpsimd.memset(qt_b[:], 0)
            nc.gpsimd.memset(kt_a[:], 0)
            nc.gpsimd.memset(kt_b[:], 0)
            nc.gpsimd.memset(vaug[:, :, :, 0:64], 0)
        if do_proj:
            kt_proj([0])()  # only chunk 0's K columns gate the first attention

        # ---- filler groups: emitted inside the ACT-bound attention k-loop
        #      so PE's idle slots do next-chunk proj / prev-chunk out-proj ----
        def qt_group(jc, m=None):
            def run():
                w = pw.tile([128, 2, QCH], f32, tag="w", name="qtp")
                for kd in range(ND):
                    nc.tensor.matmul(
                        w[:, 0, :],
                        wq[:, kd, 0:128],
                        xt[:, kd, QCH * jc : QCH * (jc + 1)],
                        start=(kd == 0),
                        stop=(kd == ND - 1),
                    )
                    nc.tensor.matmul(
                        w[0:64, 1, :],
                        wq[:, kd, 128:HS],
                        xt[:, kd, QCH * jc : QCH * (jc + 1)],
                        start=(kd == 0),
                        stop=(kd == ND - 1),
                    )
                nc.vector.tensor_scalar(
                    out=qt_a[:, QCH * jc : QCH * (jc + 1)],
                    in0=w[:, 0, :],
                    scalar1=bqa[:],
                    scalar2=None,
                    op0=add,
                )
                nc.vector.tensor_scalar(
                    out=qt_b[0:64, QCH * jc : QCH * (jc + 1)],
                    in0=w[0:64, 1, :],
                    scalar1=bqb[:],
                    scalar2=None,
                    op0=add,
                )

            return run

        def v_group(mt):
            def run():
                pv = pm.tile([128, QCH], f32, tag="s1", name="pv")
                for kd in range(ND):
                    nc.tensor.matmul(
                        pv[:, 0:HS],
                        xt[:, kd, 128 * mt : 128 * (mt + 1)],
                        wv[:, kd, :],
                        start=(kd == 0),
                        stop=(kd == ND - 1),
                    )
                nc.vector.tensor_copy(vaug[:, mt, :, 0:64], pv[:, 0:HS])

            return run

        def op_group(mt, evict_act=False):
            def run():
                ob = obp.tile([128, D], f32, name="ob")
                op = pw.tile([128, 2, QCH], f32, tag="w", name="op")
                for nh in range(2):
                    nc.tensor.matmul(
                        op[:, nh, 0:384],
                        ctxt_a[:, 128 * mt : 128 * (mt + 1)],
                        wo_a[:, 384 * nh : 384 * (nh + 1)],
                        start=True,
                        stop=False,
                    )
                for nh in range(2):
                    nc.tensor.matmul(
                        op[:, nh, 0:384],
                        ctxt_b[:, 128 * mt : 128 * (mt + 1)],
                        wo_b[:, 384 * nh : 384 * (nh + 1)],
                        start=False,
                        stop=True,
                    )
                ob2 = ob[:].rearrange("p (n m) -> p n m", n=2)
                if evict_act:
                    nc.scalar.copy(ob2, op[:, :, 0:384])
                else:
                    nc.vector.tensor_copy(ob2, op[:, :, 0:384])
                nc.sync.dma_start(
                    t["outp"].ap()[128 * mt : 128 * (mt + 1), :], ob[:]
                )

            return run

        def proj_groups(jc):
            return [v_group(mt) for mt in range(4 * jc, 4 * (jc + 1))] + [
                qt_group(jc)
            ]

        # chunk 0 projections run up front
        if do_proj:
            for g in proj_groups(0):
                g()

        # ---- attention: 3 heads as parallel pipelines, ctx one k-step
        #      behind scores so PE never head-of-line blocks on ACT ----
        def head_aps(h):
            if h < 2:
                return 64 * h, qt_a, kt_a, ctxt_a
            return 0, qt_b, kt_b, ctxt_b

        scale = float(1.0 / np.sqrt(HD))

        def norm_head(jc, cxs, h):
            # rows 64-127 of cx hold the denominator replicated: the chain is
            # just reciprocal -> multiply (no broadcast matmul needed)
            def run():
                rec = smp.tile([128, QCH], f32, tag="rec", name="rec")
                nc.vector.reciprocal(rec[64:128, :], cxs[h][64:128, :])
                row, _, _, dctx = head_aps(h)
                nc.vector.scalar_tensor_tensor(
                    out=dctx[row : row + 64, QCH * jc : QCH * (jc + 1)],
                    in0=cxs[h][0:64, :],
                    scalar=1.0,
                    in1=rec[64:128, :],
                    op0=mult,
                    op1=mult,
                )

            return run

        if do_proj and not do_attn:
            kt_proj([1, 2, 3])()
            for jc2 in range(1, NCH):
                for g in proj_groups(jc2):
                    g()
        pending = []  # deferred normalize of the previous chunk
        for jc in range(NCH) if do_attn else []:
            nkt = (QCH // KT) * (jc + 1)
            npair = nkt // 2
            cxs, ess = {}, {}

            def scores_pair(p, h):
                row, qsrc, ksrc, _ = head_aps(h)
                w = pw.tile([128, 2, QCH], f32, tag="w", name="w")
                es = esp.tile([128, 2, QCH], bf16, name="es")
                los = []
                for i in range(2):
                    kt = 2 * p + i
                    di = kt - (QCH // KT) * jc  # diagonal-band index
                    lo = 128 * di if di > 0 else 0  # valid q cols start
                    nc.tensor.matmul(
                        w[:, i, lo:QCH],
                        ksrc[row : row + 64, KT * kt : KT * (kt + 1)],
                        qsrc[row : row + 64, QCH * jc + lo : QCH * (jc + 1)],
                        start=True,
                        stop=True,
                    )
                    los.append((kt, di, lo))
                if los[0][1] >= 0 or los[1][1] >= 0:
                    # diagonal-band pair: separate column-sliced exps
                    for i, (kt, di, lo) in enumerate(los):
                        nc.scalar.activation(
                            es[:, i, lo:QCH], w[:, i, lo:QCH], Exp, scale=scale
                        )
                else:
                    # off-diagonal: one exp spanning both k-tiles
                    nc.scalar.activation(es[:, :, :], w[:, :, :], Exp, scale=scale)
                for i, (kt, di, lo) in enumerate(los):
                    if di >= 0:  # mask the triangular diagonal 128x128 block
                        # on the (otherwise idle) Pool engine: keeps the
                        # exp->ctx dependency chain off the busy DVE queue
                        nc.gpsimd.tensor_mul(
                            es[:, i, lo : lo + KT], es[:, i, lo : lo + KT], tri[:]
                        )
                ess[(p, h)] = (es, los)

            def ctx_pair(p, h):
                es, los = ess.pop((p, h))
                for i, (kt, di, lo) in enumerate(los):
                    nc.tensor.matmul(
                        cxs[h][:, lo:QCH],
                        vaug[:, kt, h, :],
                        es[:, i, lo:QCH],
                        start=(kt == 0),
                        stop=(kt == nkt - 1),
                    )

            fillers = list(pending)  # previous chunk: normalize, then out-proj
            pending = []
            if do_proj:
                if jc == 0:
                    fillers += [kt_proj([1, 2, 3])]
                if jc + 1 < NCH:
                    fillers += proj_groups(jc + 1)

            for h in range(HPC):
                cxs[h] = pc.tile([128, QCH], f32, tag="cx", name="cx")
            done = 0
            for p in range(npair):
                for h in range(HPC):
                    scores_pair(p, h)
                if p > 0:
                    for h in range(HPC):
                        ctx_pair(p - 1, h)
                want = (p + 1) * len(fillers) // npair
                while done < want:
                    fillers[done]()
                    done += 1
            for h in range(HPC):
                ctx_pair(npair - 1, h)

            pending = [norm_head(jc, cxs, h) for h in range(HPC)]
            if do_out:
                pending += [
                    op_group(mt, evict_act=(jc == NCH - 1))
                    for mt in range(4 * jc, 4 * (jc + 1))
                ]

        # last chunk's normalize + out-projection
        for g in pending:
            g()

        if not (do_proj and do_attn and do_out):
            # timing variants: consume live tensors so DCE keeps the work
            sinks = []
            if do_proj:
                sinks += [qt_a[:, 0:D], kt_a[:, 0:D], vaug[:, 0:4, :, 0:64]]
            if do_attn:
                sinks += [ctxt_a[:, 0:D], ctxt_b[0:64, 0:D]]
            for i, src in enumerate(sinks):
                snk = obp.tile([128, D], f32, name="snk")
                P = src.partition_size()
                nc.vector.tensor_copy(snk[0:P, :], src)
                nc.sync.dma_start(
                    t["outp"].ap()[128 * i : 128 * i + P, :], snk[0:P, :]
                )


def _build_program(repeat=1, phases=("proj", "attn", "out")):
    nc = bacc.Bacc(
        "TRN2", target_bir_lowering=False, debug=False, num_devices=N_CORES
    )
    bf16 = mybir.dt.bfloat16
    f32 = mybir.dt.float32
    t = {
        "xt": nc.dram_tensor("xt", [D, S], bf16, kind="ExternalInput"),
        "wq": nc.dram_tensor("wq", [D, HS], bf16, kind="ExternalInput"),
        "wk": nc.dram_tensor("wk", [D, HS], bf16, kind="ExternalInput"),
        "wv": nc.dram_tensor("wv", [D, HS], bf16, kind="ExternalInput"),
        "wo": nc.dram_tensor("wo", [HS, D], bf16, kind="ExternalInput"),
        "bq2": nc.dram_tensor("bq2", [HS, 1], f32, kind="ExternalInput"),
        "bk2": nc.dram_tensor("bk2", [HS, 1], f32, kind="ExternalInput"),
        "tri": nc.dram_tensor("tri", [128, KT], bf16, kind="ExternalInput"),
        "outp": nc.dram_tensor("outp", [S, D], f32, kind="ExternalOutput"),
    }
    with tile.TileContext(nc) as tc:
        for _ in range(repeat):
            _emit(nc, tc, t, phases)
    nc.compile()
    return nc


def _get_program(repeat=1, phases=("proj", "attn", "out")):
    key = (repeat, tuple(phases))
    if key not in _prog_cache:
        _prog_cache[key] = _build_program(repeat, phases)
    return _prog_cache[key]


def _in_maps(x, Wq, Wk, Wv, Wo, bq, bk):
    xts = [np.ascontiguousarray(x[b].T).astype(BF16) for b in range(B)]
    tri = (np.arange(KT)[None, :] >= np.arange(128)[:, None]).astype(BF16)
    maps = []
    for c in range(N_CORES):
        b, g = divmod(c, CPB)
        c0 = HS * g
        maps.append(
            {
                "xt": xts[b],
                "wq": np.ascontiguousarray(Wq[:, c0 : c0 + HS]).astype(BF16),
                "wk": np.ascontiguousarray(Wk[:, c0 : c0 + HS]).astype(BF16),
                "wv": np.ascontiguousarray(Wv[:, c0 : c0 + HS]).astype(BF16),
                "wo": np.ascontiguousarray(Wo[c0 : c0 + HS, :]).astype(BF16),
                "bq2": np.ascontiguousarray(bq[c0 : c0 + HS]).reshape(HS, 1).astype(F32),
                "bk2": np.ascontiguousarray(bk[c0 : c0 + HS]).reshape(HS, 1).astype(F32),
                "tri": tri,
            }
        )
    return maps


def kernel(x, Wq, bq, Wk, bk, Wv, bv, Wo, bo):
    x = np.asarray(x, F32)
    Wq = np.asarray(Wq, F32)
    Wk = np.asarray(Wk, F32)
    Wv = np.asarray(Wv, F32)
    Wo = np.asarray(Wo, F32)
    bq = np.asarray(bq, F32)
    bk = np.asarray(bk, F32)
    bv = np.asarray(bv, F32)
    bo = np.asarray(bo, F32)

    nc = _get_program()
    in_maps = _in_maps(x, Wq, Wk, Wv, Wo, bq, bk)

    res = bass_utils.run_bass_kernel_spmd(
        nc, in_maps, core_ids=list(range(N_CORES))
    )
    out = np.zeros((B, S, D), F32)
    for b in range(B):
        for g in range(CPB):
            out[b] += res.results[b * CPB + g]["outp"]
    out += (bv @ Wo + bo)[None, None, :]
    return out


# revision 51
# speedup vs baseline: 35.0655x; 1.0964x over previous
"""Causal self-attention (B=2, S=2048, D=768, H=12) on 8 trn2 NeuronCores.

Sharding: batch*heads = 24 head-instances -> 3 heads per core
(cores 0-3: batch 0, cores 4-7: batch 1; core c%4 owns heads 3*(c%4)..3*(c%4)+2).
Each core computes Q/K/V projections for its 192 output dims, causal
attention for its 3 heads, and a partial out-projection
ctx[2048,192] @ Wo[192,768].  Host sums the 4 partials per batch and adds
the bias terms (bo, and bv folded through Wo: softmax rows sum to 1, so
V+bv contributes exactly bv@Wo to every output row).

Device layouts (per core):
  xt   [768, 2048]  x[b]^T              bf16
  wq/wk/wv [768, 192] col slices        bf16
  wo   [192, 768]  row slice            bf16
  bq2/bk2 [192, 1] f32 (bias applied per-partition in the transposed
           Q^T/K^T layout during PSUM eviction)
  masks [4, 128, 512] multiplicative causal masks for diagonal-band tiles

Attention is computed in transposed score layout S^T[k,q] so that no PE
transposes are needed: S^T tile = K_tile @ Q_chunk^T (contraction over
head_dim on partitions), exp on ScalarE, then ctx^T accumulates via
V_aug^T @ expS^T where V_aug carries a ones column that yields the
softmax denominator for free.  Normalization happens per 64-row ctx^T
slab via a K=1 broadcast matmul of the reciprocal denominators.
"""

import sys

sys.path.insert(0, "/opt/trn_rl_repo")

import numpy as np
import ml_dtypes

import concourse.bass as bass
import concourse.bacc as bacc
import concourse.tile as tile
from concourse import mybir
from concourse import bass_utils

BF16 = ml_dtypes.bfloat16
F32 = np.float32

B, S, D, H, HD = 2, 2048, 768, 12, 64
N_CORES = 8
CPB = 4          # cores per batch element
HPC = 3          # heads per core
HS = HPC * HD    # 192 output dims per core
QCH = 512        # q chunk (columns per scores matmul)
KT = 128         # k tile
NCH = S // QCH   # 4
NKT = S // KT    # 16
NMT = S // 128   # 16 seq tiles
ND = D // 128    # 6 contraction tiles for projections

_prog_cache = {}


def _emit(nc, tc, t, phases=("proj", "attn", "out")):
    f32 = mybir.dt.float32
    bf16 = mybir.dt.bfloat16
    Exp = mybir.ActivationFunctionType.Exp
    mult = mybir.AluOpType.mult
    add = mybir.AluOpType.add

    import contextlib

    with contextlib.ExitStack() as ctx:
        sb = ctx.enter_context(tc.tile_pool(name="sb", bufs=1))
        esp = ctx.enter_context(tc.tile_pool(name="esp", bufs=8))
        smp = ctx.enter_context(tc.tile_pool(name="smp", bufs=3))
        obp = ctx.enter_context(tc.tile_pool(name="obp", bufs=2))
        # PSUM budget (8 banks): wide [128,2,512] x2 = 4 + single 1 + cx 3
        pw = ctx.enter_context(tc.tile_pool(name="pw", bufs=2, space="PSUM"))
        pm = ctx.enter_context(tc.tile_pool(name="pm", bufs=1, space="PSUM"))
        pc = ctx.enter_context(tc.tile_pool(name="pc", bufs=3, space="PSUM"))

        # ---- loads, ordered by first use; wk/xt interleaved per kd so the
        #      K projection can start accumulating as tiles arrive ----
        bka = sb.tile([128, 1], f32)
        nc.sync.dma_start(bka[:], t["bk2"].ap()[0:128, :])
        bkb = sb.tile([64, 1], f32)
        nc.sync.dma_start(bkb[:], t["bk2"].ap()[128:HS, :])
        bqa = sb.tile([128, 1], f32)
        nc.sync.dma_start(bqa[:], t["bq2"].ap()[0:128, :])
        bqb = sb.tile([64, 1], f32)
        nc.sync.dma_start(bqb[:], t["bq2"].ap()[128:HS, :])
        tri = sb.tile([128, KT], bf16)  # tri[k, q] = 1.0 if q >= k
        nc.sync.dma_start(tri[:], t["tri"].ap())

        # weights arrive pre-reshaped [128, ND*HS] (one max-descriptor DMA);
        # xt split per (kd, half) so the K projection chases the transfer
        xt = sb.tile([128, ND, S], bf16)
        wk = sb.tile([128, ND, HS], bf16)
        nc.sync.dma_start(wk[:], t["wk"].ap().rearrange("p (n m) -> p n m", n=ND))
        xt_r = t["xt"].ap().rearrange("(n p) m -> p n m", p=128)
        for kd in range(ND):
            for hf in range(2):
                sl = slice(1024 * hf, 1024 * (hf + 1))
                nc.sync.dma_start(xt[:, kd, sl], xt_r[:, kd, sl])
        wq = sb.tile([128, ND, HS], bf16)
        nc.sync.dma_start(wq[:], t["wq"].ap().rearrange("p (n m) -> p n m", n=ND))
        wv = sb.tile([128, ND, HS], bf16)
        nc.sync.dma_start(wv[:], t["wv"].ap().rearrange("p (n m) -> p n m", n=ND))
        wo_a = sb.tile([128, D], bf16)
        nc.sync.dma_start(wo_a[:], t["wo"].ap()[0:128, :])
        wo_b = sb.tile([64, D], bf16)
        nc.sync.dma_start(wo_b[:], t["wo"].ap()[128:HS, :])

        qt_a = sb.tile([128, S], bf16)   # heads 0,1 of Q^T
        qt_b = sb.tile([64, S], bf16)    # head 2 of Q^T
        kt_a = sb.tile([128, S], bf16)
        kt_b = sb.tile([64, S], bf16)
        # per (k-tile, head): [V | 64 ones columns] -> the ctx matmul yields
        # ctx^T in rows 0-63 and the softmax denominator replicated across
        # rows 64-127 (already broadcast for the normalize multiply)
        vaug = sb.tile([128, NKT, HPC, 128], bf16)
        nc.vector.memset(vaug[:, :, :, 64:128], 1.0)
        ctxt_a = sb.tile([128, S], bf16)
        ctxt_b = sb.tile([64, S], bf16)

        # ---- K^T projection helper: weights stationary per (m, kd), the
        #      given chunks' psums accumulate together ----
        def kt_proj(jcs):
            def run():
                for m in range(2):
                    P = 128 if m == 0 else 64
                    wides = [
                        pw.tile([128, 2, QCH], f32, tag="w", name="ktp")
                        for _ in range((len(jcs) + 1) // 2)
                    ]
                    slot = {
                        jc: (wides[i // 2], i % 2) for i, jc in enumerate(jcs)
                    }
                    for kd in range(ND):
                        for jc in jcs:
                            w, i = slot[jc]
                            nc.tensor.matmul(
                                w[0:P, i, :],
                                wk[:, kd, 128 * m : 128 * m + P],
                                xt[:, kd, QCH * jc : QCH * (jc + 1)],
                                start=(kd == 0),
                                stop=(kd == ND - 1),
                            )
                    dst = kt_a if m == 0 else kt_b
                    bias = bka if m == 0 else bkb
                    for jc in jcs:
                        w, i = slot[jc]
                        nc.vector.tensor_scalar(
                            out=dst[0:P, QCH * jc : QCH * (jc + 1)],
                            in0=w[0:P, i, :],
                            scalar1=bias[0:P, :],
                            scalar2=None,
                            op0=add,
                        )

            return run

        do_proj = "proj" in phases
        do_attn = "attn" in phases
        do_out = "out" in phases
        if not do_proj:
            # timing variant: skip projections, zero-fill their outputs
            nc.gpsimd.memset(qt_a[:], 0)
            nc.gpsimd.memset(qt_b[:], 0)
            nc.gpsimd.memset(kt_a[:], 0)
            nc.gpsimd.memset(kt_b[:], 0)
            nc.gpsimd.memset(vaug[:, :, :, 0:64], 0)
        if do_proj:
            kt_proj([0])()  # only chunk 0's K columns gate the first attention

        # ---- filler groups: emitted inside the ACT-bound attention k-loop
        #      so PE's idle slots do next-chunk proj / prev-chunk out-proj ----
        def qt_group(jc, m=None):
            def run():
                w = pw.tile([128, 2, QCH], f32, tag="w", name="qtp")
                for kd in range(ND):
                    nc.tensor.matmul(
                        w[:, 0, :],
                        wq[:, kd, 0:128],
                        xt[:, kd, QCH * jc : QCH * (jc + 1)],
                        start=(kd == 0),
                        stop=(kd == ND - 1),
                    )
                    nc.tensor.matmul(
                        w[0:64, 1, :],
                        wq[:, kd, 128:HS],
                        xt[:, kd, QCH * jc : QCH * (jc + 1)],
                        start=(kd == 0),
                        stop=(kd == ND - 1),
                    )
                nc.vector.tensor_scalar(
                    out=qt_a[:, QCH * jc : QCH * (jc + 1)],
                    in0=w[:, 0, :],
                    scalar1=bqa[:],
                    scalar2=None,
                    op0=add,
                )
                nc.vector.tensor_scalar(
                    out=qt_b[0:64, QCH * jc : QCH * (jc + 1)],
                    in0=w[0:64, 1, :],
                    scalar1=bqb[:],
                    scalar2=None,
                    op0=add,
                )

            return run

        def v_group(mt):
            def run():
                pv = pm.tile([128, QCH], f32, tag="s1", name="pv")
                for kd in range(ND):
                    nc.tensor.matmul(
                        pv[:, 0:HS],
                        xt[:, kd, 128 * mt : 128 * (mt + 1)],
                        wv[:, kd, :],
                        start=(kd == 0),
                        stop=(kd == ND - 1),
                    )
                nc.vector.tensor_copy(vaug[:, mt, :, 0:64], pv[:, 0:HS])

            return run

        def op_group(mt, evict_act=False):
            def run():
                ob = obp.tile([128, D], f32, name="ob")
                op = pw.tile([128, 2, QCH], f32, tag="w", name="op")
                for nh in range(2):
                    nc.tensor.matmul(
                        op[:, nh, 0:384],
                        ctxt_a[:, 128 * mt : 128 * (mt + 1)],
                        wo_a[:, 384 * nh : 384 * (nh + 1)],
                        start=True,
                        stop=False,
                    )
                for nh in range(2):
                    nc.tensor.matmul(
                        op[:, nh, 0:384],
                        ctxt_b[:, 128 * mt : 128 * (mt + 1)],
                        wo_b[:, 384 * nh : 384 * (nh + 1)],
                        start=False,
                        stop=True,
                    )
                ob2 = ob[:].rearrange("p (n m) -> p n m", n=2)
                if evict_act:
                    nc.scalar.copy(ob2, op[:, :, 0:384])
                else:
                    nc.vector.tensor_copy(ob2, op[:, :, 0:384])
                nc.sync.dma_start(
                    t["outp"].ap()[128 * mt : 128 * (mt + 1), :], ob[:]
                )

            return run

        def proj_groups(jc):
            return [v_group(mt) for mt in range(4 * jc, 4 * (jc + 1))] + [
                qt_group(jc)
            ]

        # chunk 0 projections run up front
        if do_proj:
            for g in proj_groups(0):
                g()

        # ---- attention: 3 heads as parallel pipelines, ctx one k-step
        #      behind scores so PE never head-of-line blocks on ACT ----
        def head_aps(h):
            if h < 2:
                return 64 * h, qt_a, kt_a, ctxt_a
            return 0, qt_b, kt_b, ctxt_b

        scale = float(1.0 / np.sqrt(HD))

        def norm_head(jc, cxs, h):
            # rows 64-127 of cx hold the denominator replicated: the chain is
            # just reciprocal -> multiply (no broadcast matmul needed)
            def run():
                rec = smp.tile([128, QCH], f32, tag="rec", name="rec")
                nc.vector.reciprocal(rec[64:128, :], cxs[h][64:128, :])
                row, _, _, dctx = head_aps(h)
                nc.vector.scalar_tensor_tensor(
                    out=dctx[row : row + 64, QCH * jc : QCH * (jc + 1)],
                    in0=cxs[h][0:64, :],
                    scalar=1.0,
                    in1=rec[64:128, :],
                    op0=mult,
                    op1=mult,
                )

            return run

        if do_proj and not do_attn:
            kt_proj([1, 2, 3])()
            for jc2 in range(1, NCH):
                for g in proj_groups(jc2):
                    g()
        pending = []  # deferred normalize of the previous chunk
        for jc in range(NCH) if do_attn else []:
            nkt = (QCH // KT) * (jc + 1)
            npair = nkt // 2
            cxs, ess = {}, {}

            def scores_pair(p, h):
                row, qsrc, ksrc, _ = head_aps(h)
                w = pw.tile([128, 2, QCH], f32, tag="w", name="w")
                es = esp.tile([128, 2, QCH], bf16, name="es")
                los = []
                for i in range(2):
                    kt = 2 * p + i
                    di = kt - (QCH // KT) * jc  # diagonal-band index
                    lo = 128 * di if di > 0 else 0  # valid q cols start
                    nc.tensor.matmul(
                        w[:, i, lo:QCH],
                        ksrc[row : row + 64, KT * kt : KT * (kt + 1)],
                        qsrc[row : row + 64, QCH * jc + lo : QCH * (jc + 1)],
                        start=True,
                        stop=True,
                    )
                    los.append((kt, di, lo))
                if los[0][1] >= 0 or los[1][1] >= 0:
                    # diagonal-band pair: separate column-sliced exps
                    for i, (kt, di, lo) in enumerate(los):
                        nc.scalar.activation(
                            es[:, i, lo:QCH], w[:, i, lo:QCH], Exp, scale=scale
                        )
                else:
                    # off-diagonal: one exp spanning both k-tiles
                    nc.scalar.activation(es[:, :, :], w[:, :, :], Exp, scale=scale)
                for i, (kt, di, lo) in enumerate(los):
                    if di >= 0:  # mask the triangular diagonal 128x128 block
                        # on the (otherwise idle) Pool engine: keeps the
                        # exp->ctx dependency chain off the busy DVE queue
                        nc.gpsimd.tensor_mul(
                            es[:, i, lo : lo + KT], es[:, i, lo : lo + KT], tri[:]
                        )
                ess[(p, h)] = (es, los)

            def ctx_pair(p, h):
                es, los = ess.pop((p, h))
                for i, (kt, di, lo) in enumerate(los):
                    nc.tensor.matmul(
                        cxs[h][:, lo:QCH],
                        vaug[:, kt, h, :],
                        es[:, i, lo:QCH],
                        start=(kt == 0),
                        stop=(kt == nkt - 1),
                    )

            fillers = list(pending)  # previous chunk: normalize, then out-proj
            pending = []
            if do_proj:
                if jc == 0:
                    fillers += [kt_proj([1, 2, 3])]
                if jc + 1 < NCH:
                    fillers += proj_groups(jc + 1)

            for h in range(HPC):
                cxs[h] = pc.tile([128, QCH], f32, tag="cx", name="cx")
            done = 0
            for p in range(npair):
                for h in range(HPC):
                    scores_pair(p, h)
                if p > 0:
                    for h in range(HPC):
                        ctx_pair(p - 1, h)
                want = (p + 1) * len(fillers) // npair
                while done < want:
                    fillers[done]()
                    done += 1
            for h in range(HPC):
                ctx_pair(npair - 1, h)

            pending = [norm_head(jc, cxs, h) for h in range(HPC)]
            if do_out:
                pending += [
                    op_group(mt, evict_act=(jc == NCH - 1))
                    for mt in range(4 * jc, 4 * (jc + 1))
                ]

        # last chunk's normalize + out-projection
        for g in pending:
            g()

        if not (do_proj and do_attn and do_out):
            # timing variants: consume live tensors so DCE keeps the work
            sinks = []
            if do_proj:
                sinks += [qt_a[:, 0:D], kt_a[:, 0:D], vaug[:, 0:4, :, 0:64]]
            if do_attn:
                sinks += [ctxt_a[:, 0:D], ctxt_b[0:64, 0:D]]
            for i, src in enumerate(sinks):
                snk = obp.tile([128, D], f32, name="snk")
                P = src.partition_size()
                nc.vector.tensor_copy(snk[0:P, :], src)
                nc.sync.dma_start(
                    t["outp"].ap()[128 * i : 128 * i + P, :], snk[0:P, :]
                )


def _build_program(repeat=1, phases=("proj", "attn", "out")):
    nc = bacc.Bacc(
        "TRN2", target_bir_lowering=False, debug=False, num_devices=N_CORES
    )
    bf16 = mybir.dt.bfloat16
    f32 = mybir.dt.float32
    t = {
        "xt": nc.dram_tensor("xt", [D, S], bf16, kind="ExternalInput"),
        "wq": nc.dram_tensor("wq", [128, ND * HS], bf16, kind="ExternalInput"),
        "wk": nc.dram_tensor("wk", [128, ND * HS], bf16, kind="ExternalInput"),
        "wv": nc.dram_tensor("wv", [128, ND * HS], bf16, kind="ExternalInput"),
        "wo": nc.dram_tensor("wo", [HS, D], bf16, kind="ExternalInput"),
        "bq2": nc.dram_tensor("bq2", [HS, 1], f32, kind="ExternalInput"),
        "bk2": nc.dram_tensor("bk2", [HS, 1], f32, kind="ExternalInput"),
        "tri": nc.dram_tensor("tri", [128, KT], bf16, kind="ExternalInput"),
        "outp": nc.dram_tensor("outp", [S, D], f32, kind="ExternalOutput"),
    }
    with tile.TileContext(nc) as tc:
        for _ in range(repeat):
            _emit(nc, tc, t, phases)
    nc.compile()
    return nc


def _get_program(repeat=1, phases=("proj", "attn", "out")):
    key = (repeat, tuple(phases))
    if key not in _prog_cache:
        _prog_cache[key] = _build_program(repeat, phases)
    return _prog_cache[key]


def _wr(W, c0):
    # [D, HS] slice -> [128, ND*HS]: per-partition-contiguous DMA layout
    w = np.ascontiguousarray(W[:, c0 : c0 + HS]).astype(BF16)
    return np.ascontiguousarray(
        w.reshape(ND, 128, HS).transpose(1, 0, 2).reshape(128, ND * HS)
    )


def _in_maps(x, Wq, Wk, Wv, Wo, bq, bk):
    xts = [np.ascontiguousarray(x[b].T).astype(BF16) for b in range(B)]
    tri = (np.arange(KT)[None, :] >= np.arange(128)[:, None]).astype(BF16)
    maps = []
    for c in range(N_CORES):
        b, g = divmod(c, CPB)
        c0 = HS * g
        maps.append(
            {
                "xt": xts[b],
                "wq": _wr(Wq, c0),
                "wk": _wr(Wk, c0),
                "wv": _wr(Wv, c0),
                "wo": np.ascontiguousarray(Wo[c0 : c0 + HS, :]).astype(BF16),
                "bq2": np.ascontiguousarray(bq[c0 : c0 + HS]).reshape(HS, 1).astype(F32),
                "bk2": np.ascontiguousarray(bk[c0 : c0 + HS]).reshape(HS, 1).astype(F32),
                "tri": tri,
            }
        )
    return maps


def kernel(x, Wq, bq, Wk, bk, Wv, bv, Wo, bo):
    x = np.asarray(x, F32)
    Wq = np.asarray(Wq, F32)
    Wk = np.asarray(Wk, F32)
    Wv = np.asarray(Wv, F32)
    Wo = np.asarray(Wo, F32)
    bq = np.asarray(bq, F32)
    bk = np.asarray(bk, F32)
    bv = np.asarray(bv, F32)
    bo = np.asarray(bo, F32)

    nc = _get_program()
    in_maps = _in_maps(x, Wq, Wk, Wv, Wo, bq, bk)

    res = bass_utils.run_bass_kernel_spmd(
        nc, in_maps, core_ids=list(range(N_CORES))
    )
    out = np.zeros((B, S, D), F32)
    for b in range(B):
        for g in range(CPB):
            out[b] += res.results[b * CPB + g]["outp"]
    out += (bv @ Wo + bo)[None, None, :]
    return out
